# revision 2
# baseline (speedup 1.0000x reference)
"""BasisResidualFFN Trainium2 kernel.

Math (per token t):
  recipe_soft = softmax(neuron_recipe, axis=-1)                 [64, 16]
  tr[t, :]    = sum_k w[t,k] * recipe_soft[idx[t,k], :]         [16]
  Y[t, (n,r)] = sum_d x[t,d] * basis_A[n,d,r]
  h[t, r]     = sum_n tr[t,n] * Y[t,(n,r)]
  delta[t, d] = sum_{n,r} basis_A[n,d,r] * tr[t,n] * h[t,r]
  out         = gelu((x + alpha*delta) @ w_up + b_up) @ w_down + b_down

Distribution: pure data parallel. B*S = 4096 tokens sharded 512/core
across 8 NeuronCores; all weights replicated. Everything on device is
computed feature-major (features on partitions, tokens on the free
axis, 512 tokens per matmul) so no on-device activation transposes are
needed anywhere in the FFN; x arrives pre-transposed from the host and
the output is un-transposed on the host.

Precision: the FFN runs bf16 (it dominates the error budget). The
whole basis/routing path runs fp8e4 with DoubleRow matmuls (2x PE
throughput) -- its errors enter the output only through alpha*delta
with alpha ~ 0.1, so they are strongly damped (measured 0.40% rel err
end to end vs 0.345% for all-bf16).

fp8 scale chain (all powers of two, folded into host constants):
  x8 = 16*x, a1 = 256*A1  ->  YT_psum = 4096*Y
  SEL *= 2^-12            ->  repr = tr/4096, wyt = Y*tr (bf16)
  qred *= 256             ->  ht = 256*h
  trep *= 128             ->  rh_psum = 32768*h
  ct = rh (.) repr = 8*tr*h  (fp8, |ct| < 26 << 240 = trn fp8e4 max)
  a2 = 1024*alpha*A2      ->  dl_psum = 8192*alpha*delta
  xtb = 8192*x (bf16)     ->  xf = dl + xtb = 8192*(x+alpha*delta)
  gelu(u) evaluated as Act(scale=2^-13) on u_psum = 8192*u.

Startup: PE is warmed with matmuls on a memset tile (no DMA
dependency) so the HAM clock gate ramps while the input DMAs land.
Critical loads are split across the two HWDGE rings (sync + scalar)
in need-order; late-needed tensors go on the gpsimd SWDGE ring.
"""

import numpy as np

import concourse.bass as bass
import concourse.mybir as mybir
import concourse.tile as tile
from concourse import bacc
from concourse.bass import ts
from concourse.bass_utils import run_bass_kernel_spmd

P = 128
NCORES = 8
T = 512            # tokens per core
D = 1024
DFF = 4096
NB = 16            # n_basis
R = 32             # rank
NN = 64            # n_neurons
K = 8              # top-k
DC = D // P        # 8 contraction chunks over d
FT = DFF // P      # 32 ff tiles
DT = D // P        # 8 output d tiles
NRT = (NB * R) // P  # 4 (n,r) tiles
TT = T // P        # 4 token tiles per core

# fp8 scale chain (powers of two)
SX = 16.0          # x fp8 scale
SA = 256.0         # a1 fp8 scale
S2 = 1024.0        # alpha*a2 fp8 scale
SIGR = 2.0 ** -12  # SEL scale  (= 1/(SX*SA))
SIGQ = 256.0       # qred scale
SIGT = 128.0       # trep scale (SIGT*SIGQ*SIGR = 8 = ct scale)
XS = 8192.0        # xtb prescale (= ct_scale * S2)
GS = 2.0 ** -13    # gelu input scale (= 1/XS)
F8MAX = 240.0      # trn2 fp8e4 max normal

# blob column layouts
B1_IOTA, B1_ID, B1_W = 0, 512, 640                 # bf16 blob 1
B2_SEL, B2_TREP, B2_QRED, B2_W = 0, 512, 640, 672  # bf16 blob 2
BF_BU, BF_BD, BF_REC, BF_W = 0, 32, 40, 56         # f32 blob

F32 = mybir.dt.float32
BF16 = mybir.dt.bfloat16
FP8 = mybir.dt.float8e4

NWARM = 11

_BUILT = [None]


def _build_nc():
    nc = bacc.Bacc(None, target_bir_lowering=False)

    x8_d = nc.dram_tensor("x8", [P, DC, T], FP8, kind="ExternalInput")
    xtb_d = nc.dram_tensor("xtb", [P, DC, T], BF16, kind="ExternalInput")
    idxw_d = nc.dram_tensor("idxw", [P, TT, 2 * K], BF16, kind="ExternalInput")
    blob1_d = nc.dram_tensor("blob1", [P, B1_W], BF16, kind="ExternalInput")
    blob2_d = nc.dram_tensor("blob2", [P, B2_W], BF16, kind="ExternalInput")
    blobf_d = nc.dram_tensor("blobf", [P, BF_W], F32, kind="ExternalInput")
    a1_d = nc.dram_tensor("a1", [P, DC, NB * R], FP8, kind="ExternalInput")
    a2_d = nc.dram_tensor("a2", [P, NRT, D], FP8, kind="ExternalInput")
    wu_d = nc.dram_tensor("wu", [FT // 2, P, 2, DC, P], BF16, kind="ExternalInput")
    wd_d = nc.dram_tensor("wd", [DT * 2, P, FT // 2, P], BF16, kind="ExternalInput")
    out_d = nc.dram_tensor("outT", [P, DT, T], F32, kind="ExternalOutput")

    AX = mybir.AxisListType.X
    AF = mybir.ActivationFunctionType
    ALU = mybir.AluOpType
    DR = mybir.MatmulPerfMode.DoubleRow

    with tile.TileContext(nc) as tc:
        with (
            tc.tile_pool(name="const", bufs=1) as constp,
            tc.tile_pool(name="stream", bufs=8) as stream,
            tc.tile_pool(name="otp", bufs=3) as otp,
            tc.tile_pool(name="wdstream", bufs=4) as wdstream,
            tc.tile_pool(name="mid", bufs=1) as mid,
            tc.tile_pool(name="small", bufs=2) as small,
            tc.tile_pool(name="psum", bufs=4, space="PSUM") as psum,
            tc.tile_pool(name="psums", bufs=1, space="PSUM") as psums,
        ):
            # ---- PE warm-up on a memset tile: no DMA dependency, so the
            # HAM clock gate ramps from ~7us while the input DMAs land ----
            wz = constp.tile([P, B1_W], BF16, tag="wz")
            nc.gpsimd.memset(wz[:], 0.0)
            warm_ps = psums.tile([P, T], F32, tag="htps", name="warm")
            for w in range(NWARM):
                nc.tensor.matmul(warm_ps[:], wz[:, :P], wz[:, :T],
                                 start=(w == 0), stop=(w == NWARM - 1))

            # ---- resident loads, split across DMA rings in need-order ----
            # sync HWDGE ring: YT inputs first, then the wu stream
            x8 = constp.tile([P, DC, T], FP8, tag="x8")
            a1 = constp.tile([P, DC, NB * R], FP8, tag="a1")
            h4 = ts(0, DC // 2), ts(1, DC // 2)
            nc.sync.dma_start(x8[:, h4[0], :], x8_d[:, h4[0], :])
            nc.sync.dma_start(a1[:, h4[0], :], a1_d[:, h4[0], :])
            # scalar HWDGE ring: routing inputs, second halves of YT inputs
            idxw = constp.tile([P, TT, 2 * K], BF16, tag="idxw")
            nc.scalar.dma_start(idxw[:], idxw_d[:])
            blob1 = constp.tile([P, B1_W], BF16, tag="blob1")
            nc.scalar.dma_start(blob1[:], blob1_d[:])
            nc.scalar.dma_start(x8[:, h4[1], :], x8_d[:, h4[1], :])
            blobf = constp.tile([P, BF_W], F32, tag="blobf")
            nc.scalar.dma_start(blobf[:], blobf_d[:])
            blob2 = constp.tile([P, B2_W], BF16, tag="blob2")
            nc.scalar.dma_start(blob2[:], blob2_d[:])
            nc.scalar.dma_start(a1[:, h4[1], :], a1_d[:, h4[1], :])
            # gpsimd SWDGE ring: needed only from ~18us
            a2 = constp.tile([P, NRT, D], FP8, tag="a2")
            nc.gpsimd.dma_start(a2[:], a2_d[:])
            xtb = constp.tile([P, DC, T], BF16, tag="xtb")
            nc.gpsimd.dma_start(xtb[:], xtb_d[:])

            bu = blobf[:, BF_BU:BF_BU + FT]
            bd = blobf[:, BF_BD:BF_BD + DT]
            rec = blobf[:NN, BF_REC:BF_REC + NB]
            identb = blob1[:, B1_ID:B1_ID + P]
            trep = blob2[:R, B2_TREP:B2_TREP + P]
            qred = blob2[:, B2_QRED:B2_QRED + R]

            # ---- routing: weighted one-hot scatter S[t, neuron], batched
            # over all 4 token tiles in 3 DVE ops (all-bf16 for 2x DVE) ----
            iota_b = blob1[:, B1_IOTA:B1_IOTA + NN * K].rearrange(
                "p (o n k) -> p o n k", o=1, k=K).to_broadcast((P, TT, NN, K))
            idx_b = idxw[:, :, 0:K].rearrange(
                "p t (o k) -> p t o k", o=1).to_broadcast((P, TT, NN, K))
            w_b = idxw[:, :, K:2 * K].rearrange(
                "p t (o k) -> p t o k", o=1).to_broadcast((P, TT, NN, K))
            sk = small.tile([P, TT, NN, K], BF16, tag="sk")
            nc.vector.tensor_tensor(sk[:], iota_b, idx_b, ALU.is_equal)
            nc.vector.tensor_tensor(sk[:], sk[:], w_b, ALU.mult)
            s_red = small.tile([P, TT, NN], BF16, tag="sred")
            with nc.allow_low_precision("s values are sums of <=8 weights"):
                nc.vector.reduce_sum(s_red[:], sk[:], axis=AX)
            st_sb = constp.tile([NN, T], BF16, tag="st")
            for tt in range(TT):
                stp = psums.tile([NN, P], BF16, tag="stp")
                nc.tensor.transpose(stp[:], s_red[:, tt, :], identb)
                nc.vector.tensor_copy(st_sb[:, ts(tt, P)], stp[:])

            # ---- softmax over the 16-basis axis of the recipe table (Act
            # engine; no max-subtraction needed, |recipe| is small) ----
            esb = small.tile([NN, NB], F32, tag="esb")
            ssum = small.tile([NN, 1], F32, tag="ssum")
            nc.scalar.activation(esb[:], rec, AF.Exp, accum_out=ssum[:])
            rsum = small.tile([NN, 1], F32, tag="rsum")
            nc.vector.reciprocal(rsum[:], ssum[:])
            recs = constp.tile([NN, NB], BF16, tag="recs")
            nc.scalar.activation(recs[:], esb[:], AF.Copy, scale=rsum[:, 0:1])

            # anchor read keeps the warm-up matmuls from being dead-code
            # eliminated; emitted late so it doesn't stall early DVE work
            warm_anchor = small.tile([P, 1], F32, tag="warm_anchor")
            nc.vector.tensor_copy(warm_anchor[:], warm_ps[:, 0:1])

            # token recipes, transposed: recipeT[n, t]
            rt_ps = psums.tile([NB, T], F32, tag="rtps")
            nc.tensor.matmul(rt_ps[:], recs[:], st_sb[:], start=True, stop=True)
            recipeT = constp.tile([NB, T], BF16, tag="recipeT")
            nc.vector.tensor_copy(recipeT[:], rt_ps[:])

            # RepR[(n,r), t] = SIGR * recipeT[n, t] replicated over r.
            # NOTE: uses the serially-reused "rtps" slot, NOT the "ps" ring --
            # all 4 "ps" slots hold YT results until WYT consumes them.
            repr_sb = []
            for i in range(NRT):
                rp = psums.tile([P, T], F32, tag="rtps", name=f"rp{i}")
                nc.tensor.matmul(rp[:], blob2[:NB, B2_SEL + i * P:B2_SEL + (i + 1) * P],
                                 recipeT[:], start=True, stop=True)
                rr = constp.tile([P, T], BF16, tag=f"repr{i}", name=f"repr{i}")
                nc.vector.tensor_copy(rr[:], rp[:])
                repr_sb.append(rr)

            # ---- YT = A1^T @ xT, fp8 DoubleRow (2 d-chunks per matmul) ----
            yt_ps = [psum.tile([P, T], F32, tag="ps", name=f"yt{i}")
                     for i in range(NRT)]
            for i in range(NRT):
                for cp in range(DC // 2):
                    nc.tensor.matmul(yt_ps[i][:],
                                     a1[:, 2 * cp:2 * cp + 2, ts(i, P)],
                                     x8[:, 2 * cp:2 * cp + 2, :],
                                     start=(cp == 0), stop=(cp == DC // 2 - 1),
                                     perf_mode=DR)

            # ---- WYT = YT * RepR;  hT = sum_n WYT ----
            ht_ps = psums.tile([R, T], F32, tag="htps")
            wyt = [mid.tile([P, T], BF16, tag=f"mid{i}", name=f"wyt{i}")
                   for i in range(NRT)]
            for i in range(NRT):
                nc.vector.tensor_mul(out=wyt[i][:], in0=yt_ps[i][:],
                                     in1=repr_sb[i][:])
                nc.tensor.matmul(ht_ps[:], qred, wyt[i][:],
                                 start=(i == 0), stop=(i == NRT - 1))
            ht_sb = constp.tile([R, T], BF16, tag="ht")
            nc.vector.tensor_copy(ht_sb[:], ht_ps[:])

            # ---- CT = RepH * RepR (fp8);  deltaT+x via DoubleRow ----
            rh_ps = psums.tile([P, T], F32, tag="rhps")
            nc.tensor.matmul(rh_ps[:], trep, ht_sb[:], start=True, stop=True)
            ct8 = constp.tile([P, NRT, T], FP8, tag="ct8")
            with nc.allow_low_precision("ct is alpha-damped, fp8 is enough"):
                for i in range(NRT):
                    nc.vector.tensor_mul(out=ct8[:, i, :], in0=rh_ps[:],
                                         in1=repr_sb[i][:])
            # xf = 8192*(x + alpha*delta), built in place over xtb
            for half in range(2):
                dts = range(half * 4, half * 4 + 4)
                dl_ps = {dt: psum.tile([P, T], F32, tag="ps", name=f"dl{dt}")
                         for dt in dts}
                # j outer so the first delta matmuls only need ct8[0:2]
                for j in range(NRT // 2):
                    for dt in dts:
                        nc.tensor.matmul(dl_ps[dt][:],
                                         a2[:, 2 * j:2 * j + 2, ts(dt, P)],
                                         ct8[:, 2 * j:2 * j + 2, :],
                                         start=(j == 0), stop=(j == NRT // 2 - 1),
                                         perf_mode=DR)
                for dt in dts:
                    nc.vector.tensor_add(out=xtb[:, dt, :], in0=dl_ps[dt][:],
                                         in1=xtb[:, dt, :])

            # ---- FFN up + exact gelu (descale 2^-13 folded into Act) ----
            g = constp.tile([P, FT, T], BF16, tag="g")
            for ftp in range(FT // 2):
                wu = stream.tile([P, 2, DC, P], BF16, tag="wu", name=f"wu{ftp}")
                nc.sync.dma_start(wu[:], wu_d[ftp])
                for j in range(2):
                    ft = 2 * ftp + j
                    u_ps = psum.tile([P, T], F32, tag="ps", name=f"u{ft}")
                    for dc in range(DC):
                        nc.tensor.matmul(u_ps[:], wu[:, j, dc, :], xtb[:, dc, :],
                                         start=(dc == 0), stop=(dc == DC - 1))
                    nc.scalar.activation(g[:, ft, :], u_ps[:], AF.Gelu,
                                         bias=bu[:, ft:ft + 1], scale=GS)

            # ---- FFN down + bias ----
            for dt in range(DT):
                o_ps = psum.tile([P, T], F32, tag="ps", name=f"o{dt}")
                for h in range(2):
                    wd = wdstream.tile([P, FT // 2, P], BF16, tag="wd",
                                       name=f"wd{dt}_{h}")
                    nc.scalar.dma_start(wd[:], wd_d[dt * 2 + h])
                    for fc in range(FT // 2):
                        fcg = h * (FT // 2) + fc
                        nc.tensor.matmul(o_ps[:], wd[:, fc, :], g[:, fcg, :],
                                         start=(fcg == 0), stop=(fcg == FT - 1))
                ot = otp.tile([P, T], F32, tag="ot", name=f"ot{dt}")
                nc.vector.tensor_scalar_add(ot[:], o_ps[:], bd[:, dt:dt + 1])
                nc.sync.dma_start(out_d[:, dt, :], ot[:])

    nc.finalize()
    return nc


def _get_nc():
    if _BUILT[0] is None:
        _BUILT[0] = _build_nc()
    return _BUILT[0]


def kernel(x, neuron_idx, neuron_weights, neuron_recipe, basis_A,
           w_up_w, w_up_b, w_down_w, w_down_b, alpha):
    import ml_dtypes
    nc = _get_nc()
    bf16 = ml_dtypes.bfloat16
    fp8 = ml_dtypes.float8_e4m3  # trn2 fp8e4: IEEE-style, max normal 240

    def to8(a):
        return np.clip(a, -F8MAX, F8MAX).astype(fp8)

    x = np.asarray(x, dtype=np.float32).reshape(NCORES * T, D)
    idxf = np.asarray(neuron_idx).astype(np.float32).reshape(NCORES * T, K)
    wgt = np.asarray(neuron_weights, dtype=np.float32).reshape(NCORES * T, K)
    rec = np.asarray(neuron_recipe, dtype=np.float32)
    bA = np.asarray(basis_A, dtype=np.float32)
    wu = np.asarray(w_up_w, dtype=np.float32)
    bu_in = np.asarray(w_up_b, dtype=np.float32)
    wd = np.asarray(w_down_w, dtype=np.float32)
    bd_in = np.asarray(w_down_b, dtype=np.float32)
    alpha_f = float(np.asarray(alpha, dtype=np.float32))

    # replicated operands, packed into the on-device layouts
    a1 = to8(np.ascontiguousarray(
        bA.transpose(1, 0, 2).reshape(D, NB * R)
        .reshape(DC, P, NB * R).transpose(1, 0, 2)) * SA)
    a2 = to8(np.ascontiguousarray(
        bA.transpose(0, 2, 1).reshape(NB * R, D)
        .reshape(NRT, P, D).transpose(1, 0, 2)) * (S2 * alpha_f))
    wu_p = np.ascontiguousarray(
        wu.reshape(DC, P, FT // 2, 2, P).transpose(2, 1, 3, 0, 4)
    ).astype(bf16)
    wd_p = np.ascontiguousarray(
        wd.reshape(2, FT // 2, P, DT, P).transpose(3, 0, 2, 1, 4)
        .reshape(DT * 2, P, FT // 2, P)).astype(bf16)

    blobf = np.zeros((P, BF_W), dtype=np.float32)
    blobf[:, BF_BU:BF_BU + FT] = bu_in.reshape(FT, P).T
    blobf[:, BF_BD:BF_BD + DT] = bd_in.reshape(DT, P).T
    blobf[:NN, BF_REC:BF_REC + NB] = rec

    blob1 = np.zeros((P, B1_W), dtype=np.float32)
    blob1[:, B1_IOTA:B1_IOTA + NN * K] = np.repeat(
        np.arange(NN, dtype=np.float32), K)[None, :]
    blob1[:, B1_ID:B1_ID + P] = np.eye(P, dtype=np.float32)
    blob1 = blob1.astype(bf16)

    blob2 = np.zeros((P, B2_W), dtype=np.float32)
    # SEL[n, i*128+m] = SIGR iff n == 4i + m//32
    for n in range(NB):
        i, nloc = divmod(n, NRT)
        blob2[n, B2_SEL + i * P + nloc * R: B2_SEL + i * P + (nloc + 1) * R] = SIGR
    blob2[:R, B2_TREP:B2_TREP + P] = SIGT * (
        np.arange(P)[None, :] % R == np.arange(R)[:, None])
    blob2[:, B2_QRED:B2_QRED + R] = SIGQ * (
        np.arange(P)[:, None] % R == np.arange(R)[None, :])
    blob2 = blob2.astype(bf16)

    shared = {
        "blobf": blobf, "blob1": blob1, "blob2": blob2,
        "a1": a1, "a2": a2, "wu": wu_p, "wd": wd_p,
    }
    in_maps = []
    idxw = np.concatenate([idxf, wgt], axis=1)  # [N*T, 16]
    for c in range(NCORES):
        xc = x[c * T:(c + 1) * T]  # [T, D]
        xtc = np.ascontiguousarray(xc.T.reshape(DC, P, T).transpose(1, 0, 2))
        iwc = np.ascontiguousarray(
            idxw[c * T:(c + 1) * T].reshape(TT, P, 2 * K).transpose(1, 0, 2)
        ).astype(bf16)
        in_maps.append({"x8": to8(xtc * SX), "xtb": (xtc * XS).astype(bf16),
                        "idxw": iwc, **shared})

    res = run_bass_kernel_spmd(nc, in_maps, core_ids=list(range(NCORES)))

    out = np.empty((NCORES * T, D), dtype=np.float32)
    for c in range(NCORES):
        ot = res.results[c]["outT"]  # [P, DT, T]
        out[c * T:(c + 1) * T] = ot.transpose(1, 0, 2).reshape(D, T).T
    return out.reshape(2, 2048, D)


# revision 4
# speedup vs baseline: 1.0377x; 1.0377x over previous
"""BasisResidualFFN Trainium2 kernel.

Math (per token t):
  recipe_soft = softmax(neuron_recipe, axis=-1)                 [64, 16]
  tr[t, :]    = sum_k w[t,k] * recipe_soft[idx[t,k], :]         [16]
  Y[t, (n,r)] = sum_d x[t,d] * basis_A[n,d,r]
  h[t, r]     = sum_n tr[t,n] * Y[t,(n,r)]
  delta[t, d] = sum_{n,r} basis_A[n,d,r] * tr[t,n] * h[t,r]
  out         = gelu((x + alpha*delta) @ w_up + b_up) @ w_down + b_down

Distribution: pure data parallel. B*S = 4096 tokens sharded 512/core
across 8 NeuronCores; all weights replicated. Everything on device is
computed feature-major (features on partitions, tokens on the free
axis, 512 tokens per matmul) so no on-device activation transposes are
needed anywhere in the FFN; x arrives pre-transposed from the host and
the output is un-transposed on the host.

Precision: the FFN runs bf16 (it dominates the error budget). The
whole basis/routing path runs fp8e4 with DoubleRow matmuls (2x PE
throughput) -- its errors enter the output only through alpha*delta
with alpha ~ 0.1, so they are strongly damped (measured 0.40% rel err
end to end vs 0.345% for all-bf16).

fp8 scale chain (all powers of two, folded into host constants):
  x8 = 16*x, a1 = 256*A1  ->  YT_psum = 4096*Y
  SEL *= 2^-12            ->  repr = tr/4096, wyt = Y*tr (bf16)
  qred *= 256             ->  ht = 256*h
  trep *= 128             ->  rh_psum = 32768*h
  ct = rh (.) repr = 8*tr*h  (fp8, |ct| < 26 << 240 = trn fp8e4 max)
  a2 = 1024*alpha*A2      ->  dl_psum = 8192*alpha*delta
  xtb = 8192*x (bf16)     ->  xf = dl + xtb = 8192*(x+alpha*delta)
  gelu(u) evaluated as Act(scale=2^-13) on u_psum = 8192*u.

Startup: PE is warmed with matmuls on a memset tile (no DMA
dependency) so the HAM clock gate ramps while the input DMAs land.
Critical loads are split across the two HWDGE rings (sync + scalar)
in need-order; late-needed tensors go on the gpsimd SWDGE ring.
"""

import numpy as np

import concourse.bass as bass
import concourse.mybir as mybir
import concourse.tile as tile
from concourse import bacc
from concourse.bass import ts
from concourse.bass_utils import run_bass_kernel_spmd

P = 128
NCORES = 8
T = 512            # tokens per core
D = 1024
DFF = 4096
NB = 16            # n_basis
R = 32             # rank
NN = 64            # n_neurons
K = 8              # top-k
DC = D // P        # 8 contraction chunks over d
FT = DFF // P      # 32 ff tiles
DT = D // P        # 8 output d tiles
NRT = (NB * R) // P  # 4 (n,r) tiles
TT = T // P        # 4 token tiles per core

# fp8 scale chain (powers of two)
SX = 16.0          # x fp8 scale
SA = 256.0         # a1 fp8 scale
S2 = 1024.0        # alpha*a2 fp8 scale
SIGR = 2.0 ** -12  # SEL scale  (= 1/(SX*SA))
SIGQ = 256.0       # qred scale
SIGT = 128.0       # trep scale (SIGT*SIGQ*SIGR = 8 = ct scale)
XS = 8192.0        # xtb prescale (= ct_scale * S2)
GS = 2.0 ** -13    # gelu input scale (= 1/XS)
F8MAX = 240.0      # trn2 fp8e4 max normal

# blob column layouts
B1_IOTA, B1_ID, B1_W = 0, 512, 640                 # bf16 blob 1
B2_SEL, B2_TREP, B2_QRED, B2_W = 0, 512, 640, 672  # bf16 blob 2
BF_BU, BF_BD, BF_REC, BF_W = 0, 32, 40, 56         # f32 blob

F32 = mybir.dt.float32
BF16 = mybir.dt.bfloat16
FP8 = mybir.dt.float8e4

NWARM = 10

_BUILT = [None]


def _build_nc():
    nc = bacc.Bacc(None, target_bir_lowering=False)

    x8_d = nc.dram_tensor("x8", [P, DC, T], FP8, kind="ExternalInput")
    xtb_d = nc.dram_tensor("xtb", [P, DC, T], BF16, kind="ExternalInput")
    idxw_d = nc.dram_tensor("idxw", [P, TT, 2 * K], BF16, kind="ExternalInput")
    blob1_d = nc.dram_tensor("blob1", [P, B1_W], BF16, kind="ExternalInput")
    blob2_d = nc.dram_tensor("blob2", [P, B2_W], BF16, kind="ExternalInput")
    blobf_d = nc.dram_tensor("blobf", [P, BF_W], F32, kind="ExternalInput")
    a1_d = nc.dram_tensor("a1", [P, DC, NB * R], FP8, kind="ExternalInput")
    a2_d = nc.dram_tensor("a2", [P, NRT, D], FP8, kind="ExternalInput")
    wu_d = nc.dram_tensor("wu", [FT // 2, P, 2, DC, P], BF16, kind="ExternalInput")
    wd_d = nc.dram_tensor("wd", [DT * 2, P, FT // 2, P], BF16, kind="ExternalInput")
    out_d = nc.dram_tensor("outT", [P, DT, T], F32, kind="ExternalOutput")

    AX = mybir.AxisListType.X
    AF = mybir.ActivationFunctionType
    ALU = mybir.AluOpType
    DR = mybir.MatmulPerfMode.DoubleRow

    with tile.TileContext(nc) as tc:
        with (
            tc.tile_pool(name="const", bufs=1) as constp,
            tc.tile_pool(name="stream", bufs=8) as stream,
            tc.tile_pool(name="otp", bufs=3) as otp,
            tc.tile_pool(name="wdstream", bufs=4) as wdstream,
            tc.tile_pool(name="mid", bufs=1) as mid,
            tc.tile_pool(name="small", bufs=2) as small,
            tc.tile_pool(name="psum", bufs=4, space="PSUM") as psum,
            tc.tile_pool(name="psums", bufs=1, space="PSUM") as psums,
        ):
            # ---- PE warm-up on a memset tile: no DMA dependency, so the
            # HAM clock gate ramps from ~7us while the input DMAs land ----
            wz = constp.tile([P, B1_W], BF16, tag="wz")
            nc.gpsimd.memset(wz[:], 0.0)
            warm_ps = psums.tile([P, T], F32, tag="htps", name="warm")
            for w in range(NWARM):
                nc.tensor.matmul(warm_ps[:], wz[:, :P], wz[:, :T],
                                 start=(w == 0), stop=(w == NWARM - 1))

            # ---- resident loads: strict need-order FIFO on the two HWDGE
            # rings so critical-path tensors never starve behind bulk.
            # HBM (~358 GB/s) is shared; each dma_start already stripes
            # across all 16 SDMA engines of its ring. ----
            h4 = ts(0, DC // 2), ts(1, DC // 2)
            # scalar ring: routing inputs, then a1, then x-half0
            idxw = constp.tile([P, TT, 2 * K], BF16, tag="idxw")
            nc.scalar.dma_start(idxw[:], idxw_d[:])
            blob1 = constp.tile([P, B1_W], BF16, tag="blob1")
            nc.scalar.dma_start(blob1[:], blob1_d[:])
            blobf = constp.tile([P, BF_W], F32, tag="blobf")
            nc.scalar.dma_start(blobf[:], blobf_d[:])
            blob2 = constp.tile([P, B2_W], BF16, tag="blob2")
            nc.scalar.dma_start(blob2[:], blob2_d[:])
            a1 = constp.tile([P, DC, NB * R], FP8, tag="a1")
            nc.scalar.dma_start(a1[:], a1_d[:])
            xtb = constp.tile([P, DC, T], BF16, tag="xtb")
            nc.scalar.dma_start(xtb[:, h4[0], :], xtb_d[:, h4[0], :])
            # sync ring: x8 first (YT), then a2, x-half1, then the wu stream
            x8 = constp.tile([P, DC, T], FP8, tag="x8")
            nc.sync.dma_start(x8[:], x8_d[:])
            a2 = constp.tile([P, NRT, D], FP8, tag="a2")
            nc.sync.dma_start(a2[:], a2_d[:])
            nc.sync.dma_start(xtb[:, h4[1], :], xtb_d[:, h4[1], :])

            bu = blobf[:, BF_BU:BF_BU + FT]
            bd = blobf[:, BF_BD:BF_BD + DT]
            rec = blobf[:NN, BF_REC:BF_REC + NB]
            identb = blob1[:, B1_ID:B1_ID + P]
            trep = blob2[:R, B2_TREP:B2_TREP + P]
            qred = blob2[:, B2_QRED:B2_QRED + R]

            # ---- routing: weighted one-hot scatter S[t, neuron], batched
            # over all 4 token tiles in 3 DVE ops (all-bf16 for 2x DVE) ----
            iota_b = blob1[:, B1_IOTA:B1_IOTA + NN * K].rearrange(
                "p (o n k) -> p o n k", o=1, k=K).to_broadcast((P, TT, NN, K))
            idx_b = idxw[:, :, 0:K].rearrange(
                "p t (o k) -> p t o k", o=1).to_broadcast((P, TT, NN, K))
            w_b = idxw[:, :, K:2 * K].rearrange(
                "p t (o k) -> p t o k", o=1).to_broadcast((P, TT, NN, K))
            sk = small.tile([P, TT, NN, K], BF16, tag="sk")
            nc.vector.tensor_tensor(sk[:], iota_b, idx_b, ALU.is_equal)
            nc.vector.tensor_tensor(sk[:], sk[:], w_b, ALU.mult)
            s_red = small.tile([P, TT, NN], BF16, tag="sred")
            with nc.allow_low_precision("s values are sums of <=8 weights"):
                nc.vector.reduce_sum(s_red[:], sk[:], axis=AX)
            st_sb = constp.tile([NN, T], BF16, tag="st")
            for tt in range(TT):
                stp = psums.tile([NN, P], BF16, tag="stp")
                nc.tensor.transpose(stp[:], s_red[:, tt, :], identb)
                nc.vector.tensor_copy(st_sb[:, ts(tt, P)], stp[:])

            # ---- softmax over the 16-basis axis of the recipe table (Act
            # engine; no max-subtraction needed, |recipe| is small) ----
            esb = small.tile([NN, NB], F32, tag="esb")
            ssum = small.tile([NN, 1], F32, tag="ssum")
            nc.scalar.activation(esb[:], rec, AF.Exp, accum_out=ssum[:])
            rsum = small.tile([NN, 1], F32, tag="rsum")
            nc.vector.reciprocal(rsum[:], ssum[:])
            recs = constp.tile([NN, NB], BF16, tag="recs")
            nc.scalar.activation(recs[:], esb[:], AF.Copy, scale=rsum[:, 0:1])

            # anchor read keeps the warm-up matmuls from being dead-code
            # eliminated; emitted late so it doesn't stall early DVE work
            warm_anchor = small.tile([P, 1], F32, tag="warm_anchor")
            nc.vector.tensor_copy(warm_anchor[:], warm_ps[:, 0:1])

            # token recipes, transposed: recipeT[n, t]
            rt_ps = psums.tile([NB, T], F32, tag="rtps")
            nc.tensor.matmul(rt_ps[:], recs[:], st_sb[:], start=True, stop=True)
            recipeT = constp.tile([NB, T], BF16, tag="recipeT")
            nc.vector.tensor_copy(recipeT[:], rt_ps[:])

            # RepR[(n,r), t] = SIGR * recipeT[n, t] replicated over r.
            # NOTE: uses the serially-reused "rtps" slot, NOT the "ps" ring --
            # all 4 "ps" slots hold YT results until WYT consumes them.
            repr_sb = []
            for i in range(NRT):
                rp = psums.tile([P, T], F32, tag="rtps", name=f"rp{i}")
                nc.tensor.matmul(rp[:], blob2[:NB, B2_SEL + i * P:B2_SEL + (i + 1) * P],
                                 recipeT[:], start=True, stop=True)
                rr = constp.tile([P, T], BF16, tag=f"repr{i}", name=f"repr{i}")
                nc.vector.tensor_copy(rr[:], rp[:])
                repr_sb.append(rr)

            # ---- YT = A1^T @ xT, fp8 DoubleRow (2 d-chunks per matmul) ----
            yt_ps = [psum.tile([P, T], F32, tag="ps", name=f"yt{i}")
                     for i in range(NRT)]
            for i in range(NRT):
                for cp in range(DC // 2):
                    nc.tensor.matmul(yt_ps[i][:],
                                     a1[:, 2 * cp:2 * cp + 2, ts(i, P)],
                                     x8[:, 2 * cp:2 * cp + 2, :],
                                     start=(cp == 0), stop=(cp == DC // 2 - 1),
                                     perf_mode=DR)

            # ---- WYT = YT * RepR;  hT = sum_n WYT ----
            ht_ps = psums.tile([R, T], F32, tag="htps")
            wyt = [mid.tile([P, T], BF16, tag=f"mid{i}", name=f"wyt{i}")
                   for i in range(NRT)]
            for i in range(NRT):
                nc.vector.tensor_mul(out=wyt[i][:], in0=yt_ps[i][:],
                                     in1=repr_sb[i][:])
                nc.tensor.matmul(ht_ps[:], qred, wyt[i][:],
                                 start=(i == 0), stop=(i == NRT - 1))
            ht_sb = constp.tile([R, T], BF16, tag="ht")
            nc.vector.tensor_copy(ht_sb[:], ht_ps[:])

            # ---- CT = RepH * RepR (fp8);  deltaT+x via DoubleRow ----
            rh_ps = psums.tile([P, T], F32, tag="rhps")
            nc.tensor.matmul(rh_ps[:], trep, ht_sb[:], start=True, stop=True)
            ct8 = constp.tile([P, NRT, T], FP8, tag="ct8")
            with nc.allow_low_precision("ct is alpha-damped, fp8 is enough"):
                for i in range(NRT):
                    nc.vector.tensor_mul(out=ct8[:, i, :], in0=rh_ps[:],
                                         in1=repr_sb[i][:])
            # xf = 8192*(x + alpha*delta), built in place over xtb
            for half in range(2):
                dts = range(half * 4, half * 4 + 4)
                dl_ps = {dt: psum.tile([P, T], F32, tag="ps", name=f"dl{dt}")
                         for dt in dts}
                # j outer so the first delta matmuls only need ct8[0:2]
                for j in range(NRT // 2):
                    for dt in dts:
                        nc.tensor.matmul(dl_ps[dt][:],
                                         a2[:, 2 * j:2 * j + 2, ts(dt, P)],
                                         ct8[:, 2 * j:2 * j + 2, :],
                                         start=(j == 0), stop=(j == NRT // 2 - 1),
                                         perf_mode=DR)
                for dt in dts:
                    nc.vector.tensor_add(out=xtb[:, dt, :], in0=dl_ps[dt][:],
                                         in1=xtb[:, dt, :])

            # ---- FFN up + exact gelu (descale 2^-13 folded into Act) ----
            g = constp.tile([P, FT, T], BF16, tag="g")
            for ftp in range(FT // 2):
                wu = stream.tile([P, 2, DC, P], BF16, tag="wu", name=f"wu{ftp}")
                nc.sync.dma_start(wu[:], wu_d[ftp])
                for j in range(2):
                    ft = 2 * ftp + j
                    u_ps = psum.tile([P, T], F32, tag="ps", name=f"u{ft}")
                    for dc in range(DC):
                        nc.tensor.matmul(u_ps[:], wu[:, j, dc, :], xtb[:, dc, :],
                                         start=(dc == 0), stop=(dc == DC - 1))
                    nc.scalar.activation(g[:, ft, :], u_ps[:], AF.Gelu,
                                         bias=bu[:, ft:ft + 1], scale=GS)

            # ---- FFN down + bias ----
            for dt in range(DT):
                o_ps = psum.tile([P, T], F32, tag="ps", name=f"o{dt}")
                for h in range(2):
                    wd = wdstream.tile([P, FT // 2, P], BF16, tag="wd",
                                       name=f"wd{dt}_{h}")
                    nc.scalar.dma_start(wd[:], wd_d[dt * 2 + h])
                    for fc in range(FT // 2):
                        fcg = h * (FT // 2) + fc
                        nc.tensor.matmul(o_ps[:], wd[:, fc, :], g[:, fcg, :],
                                         start=(fcg == 0), stop=(fcg == FT - 1))
                ot = otp.tile([P, T], F32, tag="ot", name=f"ot{dt}")
                nc.vector.tensor_scalar_add(ot[:], o_ps[:], bd[:, dt:dt + 1])
                nc.sync.dma_start(out_d[:, dt, :], ot[:])

    nc.finalize()
    return nc


def _get_nc():
    if _BUILT[0] is None:
        _BUILT[0] = _build_nc()
    return _BUILT[0]


def kernel(x, neuron_idx, neuron_weights, neuron_recipe, basis_A,
           w_up_w, w_up_b, w_down_w, w_down_b, alpha):
    import ml_dtypes
    nc = _get_nc()
    bf16 = ml_dtypes.bfloat16
    fp8 = ml_dtypes.float8_e4m3  # trn2 fp8e4: IEEE-style, max normal 240

    def to8(a):
        return np.clip(a, -F8MAX, F8MAX).astype(fp8)

    x = np.asarray(x, dtype=np.float32).reshape(NCORES * T, D)
    idxf = np.asarray(neuron_idx).astype(np.float32).reshape(NCORES * T, K)
    wgt = np.asarray(neuron_weights, dtype=np.float32).reshape(NCORES * T, K)
    rec = np.asarray(neuron_recipe, dtype=np.float32)
    bA = np.asarray(basis_A, dtype=np.float32)
    wu = np.asarray(w_up_w, dtype=np.float32)
    bu_in = np.asarray(w_up_b, dtype=np.float32)
    wd = np.asarray(w_down_w, dtype=np.float32)
    bd_in = np.asarray(w_down_b, dtype=np.float32)
    alpha_f = float(np.asarray(alpha, dtype=np.float32))

    # replicated operands, packed into the on-device layouts
    a1 = to8(np.ascontiguousarray(
        bA.transpose(1, 0, 2).reshape(D, NB * R)
        .reshape(DC, P, NB * R).transpose(1, 0, 2)) * SA)
    a2 = to8(np.ascontiguousarray(
        bA.transpose(0, 2, 1).reshape(NB * R, D)
        .reshape(NRT, P, D).transpose(1, 0, 2)) * (S2 * alpha_f))
    wu_p = np.ascontiguousarray(
        wu.reshape(DC, P, FT // 2, 2, P).transpose(2, 1, 3, 0, 4)
    ).astype(bf16)
    wd_p = np.ascontiguousarray(
        wd.reshape(2, FT // 2, P, DT, P).transpose(3, 0, 2, 1, 4)
        .reshape(DT * 2, P, FT // 2, P)).astype(bf16)

    blobf = np.zeros((P, BF_W), dtype=np.float32)
    blobf[:, BF_BU:BF_BU + FT] = bu_in.reshape(FT, P).T
    blobf[:, BF_BD:BF_BD + DT] = bd_in.reshape(DT, P).T
    blobf[:NN, BF_REC:BF_REC + NB] = rec

    blob1 = np.zeros((P, B1_W), dtype=np.float32)
    blob1[:, B1_IOTA:B1_IOTA + NN * K] = np.repeat(
        np.arange(NN, dtype=np.float32), K)[None, :]
    blob1[:, B1_ID:B1_ID + P] = np.eye(P, dtype=np.float32)
    blob1 = blob1.astype(bf16)

    blob2 = np.zeros((P, B2_W), dtype=np.float32)
    # SEL[n, i*128+m] = SIGR iff n == 4i + m//32
    for n in range(NB):
        i, nloc = divmod(n, NRT)
        blob2[n, B2_SEL + i * P + nloc * R: B2_SEL + i * P + (nloc + 1) * R] = SIGR
    blob2[:R, B2_TREP:B2_TREP + P] = SIGT * (
        np.arange(P)[None, :] % R == np.arange(R)[:, None])
    blob2[:, B2_QRED:B2_QRED + R] = SIGQ * (
        np.arange(P)[:, None] % R == np.arange(R)[None, :])
    blob2 = blob2.astype(bf16)

    shared = {
        "blobf": blobf, "blob1": blob1, "blob2": blob2,
        "a1": a1, "a2": a2, "wu": wu_p, "wd": wd_p,
    }
    in_maps = []
    idxw = np.concatenate([idxf, wgt], axis=1)  # [N*T, 16]
    for c in range(NCORES):
        xc = x[c * T:(c + 1) * T]  # [T, D]
        xtc = np.ascontiguousarray(xc.T.reshape(DC, P, T).transpose(1, 0, 2))
        iwc = np.ascontiguousarray(
            idxw[c * T:(c + 1) * T].reshape(TT, P, 2 * K).transpose(1, 0, 2)
        ).astype(bf16)
        in_maps.append({"x8": to8(xtc * SX), "xtb": (xtc * XS).astype(bf16),
                        "idxw": iwc, **shared})

    res = run_bass_kernel_spmd(nc, in_maps, core_ids=list(range(NCORES)))

    out = np.empty((NCORES * T, D), dtype=np.float32)
    for c in range(NCORES):
        ot = res.results[c]["outT"]  # [P, DT, T]
        out[c * T:(c + 1) * T] = ot.transpose(1, 0, 2).reshape(D, T).T
    return out.reshape(2, 2048, D)


# revision 8
# speedup vs baseline: 1.0389x; 1.0012x over previous
"""BasisResidualFFN Trainium2 kernel.

Math (per token t):
  recipe_soft = softmax(neuron_recipe, axis=-1)                 [64, 16]
  tr[t, :]    = sum_k w[t,k] * recipe_soft[idx[t,k], :]         [16]
  Y[t, (n,r)] = sum_d x[t,d] * basis_A[n,d,r]
  h[t, r]     = sum_n tr[t,n] * Y[t,(n,r)]
  delta[t, d] = sum_{n,r} basis_A[n,d,r] * tr[t,n] * h[t,r]
  out         = gelu((x + alpha*delta) @ w_up + b_up) @ w_down + b_down

Distribution: pure data parallel. B*S = 4096 tokens sharded 512/core
across 8 NeuronCores; all weights replicated. Everything on device is
computed feature-major (features on partitions, tokens on the free
axis, 512 tokens per matmul) so no on-device activation transposes are
needed anywhere in the FFN; x arrives pre-transposed from the host and
the output is un-transposed on the host.

Precision: the FFN runs bf16 (it dominates the error budget). The
basis/routing path runs fp8e4 with DoubleRow matmuls (2x PE throughput
when the HAM clock is at 8/8) -- its errors enter the output only
through alpha*delta with alpha ~ 0.1, so they are strongly damped
(measured 0.40% rel err end to end vs 0.345% for all-bf16).

fp8 scale chain (all powers of two, folded into host constants):
  x8 = 16*x, a1 = 256*A1  ->  YT_psum = 4096*Y
  SEL *= 2^-12            ->  repr = tr/4096, wyt = Y*tr (bf16)
  M = 32768*[p'%32==p%32] ->  rh_psum = sum_i M^T wyt_i = 32768*h
                              (ht and its copy are folded into one
                               accumulated matmul)
  ct = rh (.) repr = 8*tr*h  (fp8, |ct| < 26 << 240 = trn fp8e4 max)
  a2 = 1024*alpha*A2      ->  dl_psum = 8192*alpha*delta
  xtb = 8192*x (bf16)     ->  xf = Copy(dl) + xtb = 8192*(x+a*delta)
  gelu(u) evaluated as Act(scale=2^-13) on u_psum = 8192*u.

Scheduling: PE is warmed with matmuls on a memset tile (no DMA
dependency) from ~8us, and small filler matmuls keep the HAM clock
gate at 8/8 through the DVE-bound stretches of the basis phase (an
idle PE is re-throttled to half clock, which would double the cost of
everything after). PSUM->SBUF copies run on the otherwise-idle Act
engine so the DVE only does work it alone can do. DMA FIFO rings are
loaded in strict need-order.
"""

import numpy as np

import concourse.bass as bass
import concourse.mybir as mybir
import concourse.tile as tile
from concourse import bacc
from concourse.bass import ts
from concourse.bass_utils import run_bass_kernel_spmd

P = 128
NCORES = 8
T = 512            # tokens per core
D = 1024
DFF = 4096
NB = 16            # n_basis
R = 32             # rank
NN = 64            # n_neurons
K = 8              # top-k
DC = D // P        # 8 contraction chunks over d
FT = DFF // P      # 32 ff tiles
DT = D // P        # 8 output d tiles
NRT = (NB * R) // P  # 4 (n,r) tiles
TT = T // P        # 4 token tiles per core

# fp8 scale chain (powers of two)
SX = 16.0          # x fp8 scale
SA = 256.0         # a1 fp8 scale
S2 = 1024.0        # alpha*a2 fp8 scale
SIGR = 2.0 ** -12  # SEL scale  (= 1/(SX*SA))
SIGM = 32768.0     # M scale    (SIGM*SIGR = 8 = ct scale)
XS = 8192.0        # xtb prescale (= ct_scale * S2)
GS = 2.0 ** -13    # gelu input scale (= 1/XS)
F8MAX = 240.0      # trn2 fp8e4 max normal

# blob column layouts (bf16)
B1_IDX, B1_IOTA, B1_W = 0, 2 * K * TT, 2 * K * TT + NN * K  # per-core blob
B2_ID, B2_SEL, B2_QM, B2_REC, B2_W = 0, 128, 640, 768, 784
BF_BU, BF_BD, BF_W = 0, 32, 40                        # f32 biases

F32 = mybir.dt.float32
BF16 = mybir.dt.bfloat16
FP8 = mybir.dt.float8e4

NWARM = 8

_BUILT = [None]


def _build_nc():
    nc = bacc.Bacc(None, target_bir_lowering=False)

    x8_d = nc.dram_tensor("x8", [P, DC, T], FP8, kind="ExternalInput")
    xtb_d = nc.dram_tensor("xtb", [P, DC, T], BF16, kind="ExternalInput")
    blob1_d = nc.dram_tensor("blob1", [P, B1_W], BF16, kind="ExternalInput")
    blob2_d = nc.dram_tensor("blob2", [P, B2_W], BF16, kind="ExternalInput")
    blobf_d = nc.dram_tensor("blobf", [P, BF_W], F32, kind="ExternalInput")
    a1_d = nc.dram_tensor("a1", [P, DC, NB * R], FP8, kind="ExternalInput")
    a2_d = nc.dram_tensor("a2", [P, NRT, D], FP8, kind="ExternalInput")
    wu_d = nc.dram_tensor("wu", [FT // 2, P, 2, DC, P], BF16, kind="ExternalInput")
    wd_d = nc.dram_tensor("wd", [DT * 2, P, FT // 2, P], BF16, kind="ExternalInput")
    out_d = nc.dram_tensor("outT", [P, DT, T], F32, kind="ExternalOutput")

    AX = mybir.AxisListType.X
    AF = mybir.ActivationFunctionType
    ALU = mybir.AluOpType
    DR = mybir.MatmulPerfMode.DoubleRow

    with tile.TileContext(nc) as tc:
        with (
            tc.tile_pool(name="const", bufs=1) as constp,
            tc.tile_pool(name="stream", bufs=4) as stream,
            tc.tile_pool(name="otp", bufs=3) as otp,
            tc.tile_pool(name="wdstream", bufs=4) as wdstream,
            tc.tile_pool(name="mid", bufs=1) as mid,
            tc.tile_pool(name="small", bufs=2) as small,
            tc.tile_pool(name="tmpp", bufs=3) as tmpp,
            tc.tile_pool(name="psum", bufs=4, space="PSUM") as psum,
            tc.tile_pool(name="psums", bufs=1, space="PSUM") as psums,
        ):
            # ---- PE warm-up on a memset tile: no DMA dependency, so the
            # HAM clock gate ramps from ~8us while the input DMAs land ----
            wz = constp.tile([P, T + P], BF16, tag="wz")
            nc.gpsimd.memset(wz[:], 0.0)
            warm_ps = psums.tile([P, T], F32, tag="htps", name="warm")
            for w in range(NWARM):
                nc.tensor.matmul(warm_ps[:], wz[:, :P], wz[:, :T],
                                 start=(w == 0), stop=(w == NWARM - 1))

            def filler(name, n):
                # small matmuls that keep the HAM clock gate at 8/8 while
                # the PE waits on DVE stages; drained in ~100ns each once
                # real work unblocks.  Anchored via the idle Act engine.
                f_ps = psums.tile([P, T], F32, tag="htps", name=f"fill_{name}")
                for i in range(n):
                    nc.tensor.matmul(f_ps[:, :P], wz[:, :P], wz[:, T:T + P],
                                     start=(i == 0), stop=(i == n - 1))
                fa = tmpp.tile([P, 1], F32, tag="fanchor", name=f"fa_{name}")
                nc.scalar.activation(fa[:], f_ps[:, 0:1], AF.Copy)

            # ---- resident loads: strict need-order FIFO on the two HWDGE
            # rings so critical-path tensors never starve behind bulk ----
            # scalar ring: routing blobs, a1, then xtb and biases
            blob1 = constp.tile([P, B1_W], BF16, tag="blob1")
            nc.scalar.dma_start(blob1[:], blob1_d[:])
            blob2 = constp.tile([P, B2_W], BF16, tag="blob2")
            nc.scalar.dma_start(blob2[:], blob2_d[:])
            a1 = constp.tile([P, DC, NB * R], FP8, tag="a1")
            nc.scalar.dma_start(a1[:], a1_d[:])
            xtb = constp.tile([P, DC, T], BF16, tag="xtb")
            nc.scalar.dma_start(xtb[:], xtb_d[:])
            blobf = constp.tile([P, BF_W], F32, tag="blobf")
            nc.scalar.dma_start(blobf[:], blobf_d[:])
            # sync ring: x8 first (YT), then a2, then the wu stream
            x8 = constp.tile([P, DC, T], FP8, tag="x8")
            nc.sync.dma_start(x8[:], x8_d[:])
            a2 = constp.tile([P, NRT, D], FP8, tag="a2")
            nc.sync.dma_start(a2[:], a2_d[:])

            bu = blobf[:, BF_BU:BF_BU + FT]
            bd = blobf[:, BF_BD:BF_BD + DT]
            identb = blob2[:, B2_ID:B2_ID + P]
            qm = blob2[:, B2_QM:B2_QM + P]
            rec = blob2[:NN, B2_REC:B2_REC + NB]

            # ---- routing: weighted one-hot scatter S[t, neuron], batched
            # over all 4 token tiles in 3 DVE ops (all-bf16 for 2x DVE) ----
            iota_b = blob1[:, B1_IOTA:B1_IOTA + NN * K].rearrange(
                "p (o n k) -> p o n k", o=1, k=K).to_broadcast((P, TT, NN, K))
            idxw = blob1[:, B1_IDX:B1_IDX + 2 * K * TT].rearrange(
                "p (t k) -> p t k", t=TT)
            idx_b = idxw[:, :, 0:K].rearrange(
                "p t (o k) -> p t o k", o=1).to_broadcast((P, TT, NN, K))
            w_b = idxw[:, :, K:2 * K].rearrange(
                "p t (o k) -> p t o k", o=1).to_broadcast((P, TT, NN, K))
            sk = small.tile([P, TT, NN, K], BF16, tag="sk")
            nc.vector.tensor_tensor(sk[:], iota_b, idx_b, ALU.is_equal)
            nc.vector.tensor_tensor(sk[:], sk[:], w_b, ALU.mult)
            s_red = small.tile([P, TT, NN], BF16, tag="sred")
            with nc.allow_low_precision("s values are sums of <=8 weights"):
                nc.vector.reduce_sum(s_red[:], sk[:], axis=AX)

            # ---- softmax over the 16-basis axis of the recipe table ----
            esb = small.tile([NN, NB], F32, tag="esb")
            ssum = small.tile([NN, 1], F32, tag="ssum")
            nc.scalar.activation(esb[:], rec, AF.Exp, accum_out=ssum[:])
            rsum = small.tile([NN, 1], F32, tag="rsum")
            nc.vector.reciprocal(rsum[:], ssum[:])
            recs = constp.tile([NN, NB], BF16, tag="recs")
            nc.scalar.activation(recs[:], esb[:], AF.Copy, scale=rsum[:, 0:1])

            # anchor read keeps the warm-up matmuls from being dead-code
            # eliminated (Act engine; DVE is the busy one here)
            warm_anchor = tmpp.tile([P, 1], F32, tag="fanchor", name="wanchor")
            nc.scalar.activation(warm_anchor[:], warm_ps[:, 0:1], AF.Copy)

            # ---- YT = A1^T @ xT, fp8 DoubleRow (2 d-chunks per matmul) ----
            yt_ps = [psum.tile([P, T], F32, tag="ps", name=f"yt{i}")
                     for i in range(NRT)]
            for i in range(NRT):
                for cp in range(DC // 2):
                    nc.tensor.matmul(yt_ps[i][:],
                                     a1[:, 2 * cp:2 * cp + 2, ts(i, P)],
                                     x8[:, 2 * cp:2 * cp + 2, :],
                                     start=(cp == 0), stop=(cp == DC // 2 - 1),
                                     perf_mode=DR)
            filler("a", 12)

            # scatter transposes + token recipes, transposed: recipeT[n, t]
            st_sb = constp.tile([NN, T], BF16, tag="st")
            for tt in range(TT):
                stp = psums.tile([NN, P], BF16, tag="stp")
                nc.tensor.transpose(stp[:], s_red[:, tt, :], identb)
                nc.vector.tensor_copy(st_sb[:, ts(tt, P)], stp[:])
            rt_ps = psums.tile([NB, T], F32, tag="rtps")
            nc.tensor.matmul(rt_ps[:], recs[:], st_sb[:], start=True, stop=True)
            recipeT = constp.tile([NB, T], BF16, tag="recipeT")
            nc.vector.tensor_copy(recipeT[:], rt_ps[:])

            # RepR[(n,r), t] = SIGR * recipeT[n, t] replicated over r.
            # PSUM->SBUF copies go on the idle Act engine, not the DVE.
            repr_sb = []
            for i in range(NRT):
                rp = psums.tile([P, T], F32, tag="rtps", name=f"rp{i}")
                nc.tensor.matmul(rp[:], blob2[:NB, B2_SEL + i * P:B2_SEL + (i + 1) * P],
                                 recipeT[:], start=True, stop=True)
                rr = constp.tile([P, T], BF16, tag=f"repr{i}", name=f"repr{i}")
                nc.scalar.activation(rr[:], rp[:], AF.Copy)
                repr_sb.append(rr)
            filler("b", 12)

            # ---- WYT = YT * RepR;  rh = 32768*h via one accumulated matmul
            # (M folds the n-sum AND the r-replication: M[p',p] =
            #  32768*[p'%32 == p%32]) ----
            rh_ps = psums.tile([P, T], F32, tag="rhps")
            wyt = [mid.tile([P, T], BF16, tag=f"mid{i}", name=f"wyt{i}")
                   for i in range(NRT)]
            for i in range(NRT):
                nc.vector.tensor_mul(out=wyt[i][:], in0=yt_ps[i][:],
                                     in1=repr_sb[i][:])
                nc.tensor.matmul(rh_ps[:], qm, wyt[i][:],
                                 start=(i == 0), stop=(i == NRT - 1))
            filler("c", 8)

            # ---- CT = RepH * RepR (fp8);  delta via DoubleRow ----
            ct8 = constp.tile([P, NRT, T], FP8, tag="ct8")
            with nc.allow_low_precision("ct is alpha-damped, fp8 is enough"):
                for i in range(NRT):
                    nc.vector.tensor_mul(out=ct8[:, i, :], in0=rh_ps[:],
                                         in1=repr_sb[i][:])
            # xf = 8192*(x + alpha*delta): Act copies psum -> bf16, DVE adds
            for half in range(2):
                dts = range(half * 4, half * 4 + 4)
                dl_ps = {dt: psum.tile([P, T], F32, tag="ps", name=f"dl{dt}")
                         for dt in dts}
                # j outer so the first delta matmuls only need ct8[0:2]
                for j in range(NRT // 2):
                    for dt in dts:
                        nc.tensor.matmul(dl_ps[dt][:],
                                         a2[:, 2 * j:2 * j + 2, ts(dt, P)],
                                         ct8[:, 2 * j:2 * j + 2, :],
                                         start=(j == 0), stop=(j == NRT // 2 - 1),
                                         perf_mode=DR)
                for dt in dts:
                    dl_sb = tmpp.tile([P, T], BF16, tag="dl", name=f"dl_sb{dt}")
                    nc.scalar.activation(dl_sb[:], dl_ps[dt][:], AF.Copy)
                    nc.vector.tensor_add(out=xtb[:, dt, :], in0=dl_sb[:],
                                         in1=xtb[:, dt, :])
            filler("e", 8)

            # ---- FFN up + exact gelu (descale 2^-13 folded into Act) ----
            g = constp.tile([P, FT, T], BF16, tag="g")
            for ftp in range(FT // 2):
                wu = stream.tile([P, 2, DC, P], BF16, tag="wu", name=f"wu{ftp}")
                nc.sync.dma_start(wu[:], wu_d[ftp])
                for j in range(2):
                    ft = 2 * ftp + j
                    u_ps = psum.tile([P, T], F32, tag="ps", name=f"u{ft}")
                    for dc in range(DC):
                        nc.tensor.matmul(u_ps[:], wu[:, j, dc, :], xtb[:, dc, :],
                                         start=(dc == 0), stop=(dc == DC - 1))
                    nc.scalar.activation(g[:, ft, :], u_ps[:], AF.Gelu,
                                         bias=bu[:, ft:ft + 1], scale=GS)

            # ---- FFN down + bias ----
            for dt in range(DT):
                o_ps = psum.tile([P, T], F32, tag="ps", name=f"o{dt}")
                for h in range(2):
                    wd = wdstream.tile([P, FT // 2, P], BF16, tag="wd",
                                       name=f"wd{dt}_{h}")
                    nc.scalar.dma_start(wd[:], wd_d[dt * 2 + h])
                    for fc in range(FT // 2):
                        fcg = h * (FT // 2) + fc
                        nc.tensor.matmul(o_ps[:], wd[:, fc, :], g[:, fcg, :],
                                         start=(fcg == 0), stop=(fcg == FT - 1))
                ot = otp.tile([P, T], F32, tag="ot", name=f"ot{dt}")
                nc.vector.tensor_scalar_add(ot[:], o_ps[:], bd[:, dt:dt + 1])
                nc.sync.dma_start(out_d[:, dt, :], ot[:])

    nc.finalize()
    return nc


def _get_nc():
    if _BUILT[0] is None:
        _BUILT[0] = _build_nc()
    return _BUILT[0]


def kernel(x, neuron_idx, neuron_weights, neuron_recipe, basis_A,
           w_up_w, w_up_b, w_down_w, w_down_b, alpha):
    import ml_dtypes
    nc = _get_nc()
    bf16 = ml_dtypes.bfloat16
    fp8 = ml_dtypes.float8_e4m3  # trn2 fp8e4: IEEE-style, max normal 240

    def to8(a):
        return np.clip(a, -F8MAX, F8MAX).astype(fp8)

    x = np.asarray(x, dtype=np.float32).reshape(NCORES * T, D)
    idxf = np.asarray(neuron_idx).astype(np.float32).reshape(NCORES * T, K)
    wgt = np.asarray(neuron_weights, dtype=np.float32).reshape(NCORES * T, K)
    rec = np.asarray(neuron_recipe, dtype=np.float32)
    bA = np.asarray(basis_A, dtype=np.float32)
    wu = np.asarray(w_up_w, dtype=np.float32)
    bu_in = np.asarray(w_up_b, dtype=np.float32)
    wd = np.asarray(w_down_w, dtype=np.float32)
    bd_in = np.asarray(w_down_b, dtype=np.float32)
    alpha_f = float(np.asarray(alpha, dtype=np.float32))

    # replicated operands, packed into the on-device layouts
    a1 = to8(np.ascontiguousarray(
        bA.transpose(1, 0, 2).reshape(D, NB * R)
        .reshape(DC, P, NB * R).transpose(1, 0, 2)) * SA)
    a2 = to8(np.ascontiguousarray(
        bA.transpose(0, 2, 1).reshape(NB * R, D)
        .reshape(NRT, P, D).transpose(1, 0, 2)) * (S2 * alpha_f))
    wu_p = np.ascontiguousarray(
        wu.reshape(DC, P, FT // 2, 2, P).transpose(2, 1, 3, 0, 4)
    ).astype(bf16)
    wd_p = np.ascontiguousarray(
        wd.reshape(2, FT // 2, P, DT, P).transpose(3, 0, 2, 1, 4)
        .reshape(DT * 2, P, FT // 2, P)).astype(bf16)

    blobf = np.zeros((P, BF_W), dtype=np.float32)
    blobf[:, BF_BU:BF_BU + FT] = bu_in.reshape(FT, P).T
    blobf[:, BF_BD:BF_BD + DT] = bd_in.reshape(DT, P).T

    blob2 = np.zeros((P, B2_W), dtype=np.float32)
    blob2[:, B2_ID:B2_ID + P] = np.eye(P, dtype=np.float32)
    # SEL[n, i*128+m] = SIGR iff n == 4i + m//32
    for n in range(NB):
        i, nloc = divmod(n, NRT)
        blob2[n, B2_SEL + i * P + nloc * R: B2_SEL + i * P + (nloc + 1) * R] = SIGR
    blob2[:, B2_QM:B2_QM + P] = SIGM * (
        np.arange(P)[:, None] % R == np.arange(P)[None, :] % R)
    blob2[:NN, B2_REC:B2_REC + NB] = rec
    blob2 = blob2.astype(bf16)

    shared = {
        "blobf": blobf, "blob2": blob2,
        "a1": a1, "a2": a2, "wu": wu_p, "wd": wd_p,
    }
    in_maps = []
    idxw = np.concatenate([idxf, wgt], axis=1)  # [N*T, 16]
    blob1_base = np.zeros((P, B1_W), dtype=np.float32)
    blob1_base[:, B1_IOTA:B1_IOTA + NN * K] = np.repeat(
        np.arange(NN, dtype=np.float32), K)[None, :]
    for c in range(NCORES):
        xc = x[c * T:(c + 1) * T]  # [T, D]
        xtc = np.ascontiguousarray(xc.T.reshape(DC, P, T).transpose(1, 0, 2))
        blob1 = blob1_base.copy()
        blob1[:, B1_IDX:B1_IDX + 2 * K * TT] = (
            idxw[c * T:(c + 1) * T].reshape(TT, P, 2 * K).transpose(1, 0, 2)
            .reshape(P, 2 * K * TT))
        in_maps.append({"x8": to8(xtc * SX), "xtb": (xtc * XS).astype(bf16),
                        "blob1": blob1.astype(bf16), **shared})

    res = run_bass_kernel_spmd(nc, in_maps, core_ids=list(range(NCORES)))

    out = np.empty((NCORES * T, D), dtype=np.float32)
    for c in range(NCORES):
        ot = res.results[c]["outT"]  # [P, DT, T]
        out[c * T:(c + 1) * T] = ot.transpose(1, 0, 2).reshape(D, T).T
    return out.reshape(2, 2048, D)


# revision 10
# speedup vs baseline: 1.0630x; 1.0232x over previous
"""BasisResidualFFN Trainium2 kernel.

Math (per token t):
  recipe_soft = softmax(neuron_recipe, axis=-1)                 [64, 16]
  tr[t, :]    = sum_k w[t,k] * recipe_soft[idx[t,k], :]         [16]
  Y[t, (n,r)] = sum_d x[t,d] * basis_A[n,d,r]
  h[t, r]     = sum_n tr[t,n] * Y[t,(n,r)]
  delta[t, d] = sum_{n,r} basis_A[n,d,r] * tr[t,n] * h[t,r]
  out         = gelu((x + alpha*delta) @ w_up + b_up) @ w_down + b_down

Distribution: pure data parallel. B*S = 4096 tokens sharded 512/core
across 8 NeuronCores; all weights replicated. Everything on device is
computed feature-major (features on partitions, tokens on the free
axis, 512 tokens per matmul) so no on-device activation transposes are
needed anywhere in the FFN; x arrives pre-transposed from the host and
the output is un-transposed on the host.

Precision: the FFN runs bf16 (it dominates the error budget). The
basis/routing path runs fp8e4 with DoubleRow matmuls (2x PE throughput
when the HAM clock is at 8/8) -- its errors enter the output only
through alpha*delta with alpha ~ 0.1, so they are strongly damped
(measured 0.40% rel err end to end vs 0.345% for all-bf16).

fp8 scale chain (all powers of two, folded into host constants):
  x8 = 16*x, a1 = 256*A1  ->  YT_psum = 4096*Y
  SEL *= 2^-12            ->  repr = tr/4096, wyt = Y*tr (bf16)
  M = 32768*[p'%32==p%32] ->  rh_psum = sum_i M^T wyt_i = 32768*h
                              (ht and its copy are folded into one
                               accumulated matmul)
  ct = rh (.) repr = 8*tr*h  (fp8, |ct| < 26 << 240 = trn fp8e4 max)
  a2 = 1024*alpha*A2      ->  dl_psum = 8192*alpha*delta
  xtb = 8192*x (bf16)     ->  xf = Copy(dl) + xtb = 8192*(x+a*delta)
  gelu(u) evaluated as Act(scale=2^-13) on u_psum = 8192*u.

Scheduling: PE is warmed with matmuls on a memset tile (no DMA
dependency) from ~8us, and small filler matmuls keep the HAM clock
gate at 8/8 through the DVE-bound stretches of the basis phase (an
idle PE is re-throttled to half clock, which would double the cost of
everything after). PSUM->SBUF copies are split between the Act engine
and the DVE so neither serializes the chain. The two HWDGE DMA rings
are loaded in strict need-order (each dma_start costs ~2us fixed +
bytes/rate, FIFO per ring), with all small constants merged into one
per-core blob.
"""

import numpy as np

import concourse.bass as bass
import concourse.mybir as mybir
import concourse.tile as tile
from concourse import bacc
from concourse.bass import ts
from concourse.bass_utils import run_bass_kernel_spmd

P = 128
NCORES = 8
T = 512            # tokens per core
D = 1024
DFF = 4096
NB = 16            # n_basis
R = 32             # rank
NN = 64            # n_neurons
K = 8              # top-k
DC = D // P        # 8 contraction chunks over d
FT = DFF // P      # 32 ff tiles
DT = D // P        # 8 output d tiles
NRT = (NB * R) // P  # 4 (n,r) tiles
TT = T // P        # 4 token tiles per core

# fp8 scale chain (powers of two)
SX = 16.0          # x fp8 scale
SA = 256.0         # a1 fp8 scale
S2 = 1024.0        # alpha*a2 fp8 scale
SIGR = 2.0 ** -12  # SEL scale  (= 1/(SX*SA))
SIGM = 32768.0     # M scale    (SIGM*SIGR = 8 = ct scale)
XS = 8192.0        # xtb prescale (= ct_scale * S2)
GS = 2.0 ** -13    # gelu input scale (= 1/XS)
F8MAX = 240.0      # trn2 fp8e4 max normal

# merged bf16 blob column layout (per-core: contains idx/weights)
BB_IDX = 0
BB_IOTA = BB_IDX + 2 * K * TT      # 64
BB_ID = BB_IOTA + NN * K           # 576
BB_SEL = BB_ID + P                 # 704
BB_QM = BB_SEL + NRT * P           # 1216
BB_REC = BB_QM + P                 # 1344
BB_W = BB_REC + NB                 # 1360
BF_BU, BF_BD, BF_W = 0, 32, 40     # f32 biases

F32 = mybir.dt.float32
BF16 = mybir.dt.bfloat16
FP8 = mybir.dt.float8e4

NWARM = 11

_BUILT = [None]


def _build_nc():
    nc = bacc.Bacc(None, target_bir_lowering=False)

    x8_d = nc.dram_tensor("x8", [P, DC, T], FP8, kind="ExternalInput")
    xtb_d = nc.dram_tensor("xtb", [P, DC, T], BF16, kind="ExternalInput")
    blobb_d = nc.dram_tensor("blobb", [P, BB_W], BF16, kind="ExternalInput")
    blobf_d = nc.dram_tensor("blobf", [P, BF_W], F32, kind="ExternalInput")
    a1_d = nc.dram_tensor("a1", [P, DC, NB * R], FP8, kind="ExternalInput")
    a2_d = nc.dram_tensor("a2", [P, NRT, D], FP8, kind="ExternalInput")
    wu_d = nc.dram_tensor("wu", [FT // 2, P, 2, DC, P], BF16, kind="ExternalInput")
    wd_d = nc.dram_tensor("wd", [DT * 2, P, FT // 2, P], BF16, kind="ExternalInput")
    out_d = nc.dram_tensor("outT", [P, DT, T], F32, kind="ExternalOutput")

    AX = mybir.AxisListType.X
    AF = mybir.ActivationFunctionType
    ALU = mybir.AluOpType
    DR = mybir.MatmulPerfMode.DoubleRow

    with tile.TileContext(nc) as tc:
        with (
            tc.tile_pool(name="const", bufs=1) as constp,
            tc.tile_pool(name="stream", bufs=4) as stream,
            tc.tile_pool(name="otp", bufs=3) as otp,
            tc.tile_pool(name="wdstream", bufs=4) as wdstream,
            tc.tile_pool(name="mid", bufs=1) as mid,
            tc.tile_pool(name="small", bufs=2) as small,
            tc.tile_pool(name="tmpp", bufs=3) as tmpp,
            tc.tile_pool(name="psum", bufs=4, space="PSUM") as psum,
            tc.tile_pool(name="psums", bufs=1, space="PSUM") as psums,
        ):
            # ---- PE warm-up on a memset tile: no DMA dependency, so the
            # HAM clock gate ramps from ~8us while the input DMAs land ----
            wz = constp.tile([P, T + P], BF16, tag="wz")
            nc.gpsimd.memset(wz[:], 0.0)
            warm_ps = psums.tile([P, T], F32, tag="htps", name="warm")
            for w in range(NWARM):
                nc.tensor.matmul(warm_ps[:], wz[:, :P], wz[:, :T],
                                 start=(w == 0), stop=(w == NWARM - 1))

            def filler(name, n):
                # small matmuls that keep the HAM clock gate at 8/8 while
                # the PE waits on DVE stages; drained in ~110ns each once
                # real work unblocks.  Anchored via the idle Act engine.
                f_ps = psums.tile([P, T], F32, tag="htps", name=f"fill_{name}")
                for i in range(n):
                    nc.tensor.matmul(f_ps[:, :P], wz[:, :P], wz[:, T:T + P],
                                     start=(i == 0), stop=(i == n - 1))
                fa = tmpp.tile([P, 1], F32, tag="fanchor", name=f"fa_{name}")
                nc.scalar.activation(fa[:], f_ps[:, 0:1], AF.Copy)

            # ---- resident loads: strict need-order FIFO on the two HWDGE
            # rings so critical-path tensors never starve behind bulk ----
            # sync ring: routing blob first (gates the DVE scatter chain),
            # then x8 (YT), a2, biases, then the wu stream
            blobb = constp.tile([P, BB_W], BF16, tag="blobb")
            nc.sync.dma_start(blobb[:], blobb_d[:])
            x8 = constp.tile([P, DC, T], FP8, tag="x8")
            nc.sync.dma_start(x8[:], x8_d[:])
            a2 = constp.tile([P, NRT, D], FP8, tag="a2")
            nc.sync.dma_start(a2[:], a2_d[:])
            blobf = constp.tile([P, BF_W], F32, tag="blobf")
            nc.sync.dma_start(blobf[:], blobf_d[:])
            # scalar ring: a1 (YT stationary), then xtb, later the wd stream
            a1 = constp.tile([P, DC, NB * R], FP8, tag="a1")
            nc.scalar.dma_start(a1[:], a1_d[:])
            xtb = constp.tile([P, DC, T], BF16, tag="xtb")
            nc.scalar.dma_start(xtb[:], xtb_d[:])

            bu = blobf[:, BF_BU:BF_BU + FT]
            bd = blobf[:, BF_BD:BF_BD + DT]
            identb = blobb[:, BB_ID:BB_ID + P]
            qm = blobb[:, BB_QM:BB_QM + P]
            rec = blobb[:NN, BB_REC:BB_REC + NB]

            # ---- routing: weighted one-hot scatter S[t, neuron], batched
            # over all 4 token tiles (all-bf16 for 2x DVE); the K-reduction
            # is split DVE/GpSimd ----
            iota_b = blobb[:, BB_IOTA:BB_IOTA + NN * K].rearrange(
                "p (o n k) -> p o n k", o=1, k=K).to_broadcast((P, TT, NN, K))
            idxw = blobb[:, BB_IDX:BB_IDX + 2 * K * TT].rearrange(
                "p (t k) -> p t k", t=TT)
            idx_b = idxw[:, :, 0:K].rearrange(
                "p t (o k) -> p t o k", o=1).to_broadcast((P, TT, NN, K))
            w_b = idxw[:, :, K:2 * K].rearrange(
                "p t (o k) -> p t o k", o=1).to_broadcast((P, TT, NN, K))
            sk = small.tile([P, TT, NN, K], BF16, tag="sk")
            nc.vector.tensor_tensor(sk[:], iota_b, idx_b, ALU.is_equal)
            nc.vector.tensor_tensor(sk[:], sk[:], w_b, ALU.mult)
            s_red = small.tile([P, TT, NN], BF16, tag="sred")
            with nc.allow_low_precision("s values are sums of <=8 weights"):
                nc.vector.reduce_sum(s_red[:, 0:2, :], sk[:, 0:2, :, :], axis=AX)
                # GpSimd takes the other half via pairwise adds (it has no
                # free-axis reduce)
                nc.gpsimd.tensor_tensor(sk[:, 2:4, :, 0:4], sk[:, 2:4, :, 0:4],
                                        sk[:, 2:4, :, 4:8], ALU.add)
                nc.gpsimd.tensor_tensor(sk[:, 2:4, :, 0:2], sk[:, 2:4, :, 0:2],
                                        sk[:, 2:4, :, 2:4], ALU.add)
                nc.gpsimd.tensor_tensor(s_red[:, 2:4, :], sk[:, 2:4, :, 0],
                                        sk[:, 2:4, :, 1], ALU.add)

            # ---- softmax over the 16-basis axis of the recipe table ----
            esb = small.tile([NN, NB], F32, tag="esb")
            ssum = small.tile([NN, 1], F32, tag="ssum")
            nc.scalar.activation(esb[:], rec, AF.Exp, accum_out=ssum[:])
            rsum = small.tile([NN, 1], F32, tag="rsum")
            nc.vector.reciprocal(rsum[:], ssum[:])
            recs = constp.tile([NN, NB], BF16, tag="recs")
            nc.scalar.activation(recs[:], esb[:], AF.Copy, scale=rsum[:, 0:1])

            # anchor read keeps the warm-up matmuls from being dead-code
            # eliminated (Act engine; DVE is the busy one here)
            warm_anchor = tmpp.tile([P, 1], F32, tag="fanchor", name="wanchor")
            nc.scalar.activation(warm_anchor[:], warm_ps[:, 0:1], AF.Copy)

            # ---- YT = A1^T @ xT, fp8 DoubleRow (2 d-chunks per matmul) ----
            yt_ps = [psum.tile([P, T], F32, tag="ps", name=f"yt{i}")
                     for i in range(NRT)]
            for i in range(NRT):
                for cp in range(DC // 2):
                    nc.tensor.matmul(yt_ps[i][:],
                                     a1[:, 2 * cp:2 * cp + 2, ts(i, P)],
                                     x8[:, 2 * cp:2 * cp + 2, :],
                                     start=(cp == 0), stop=(cp == DC // 2 - 1),
                                     perf_mode=DR)
            filler("a", 6)

            # scatter transposes + token recipes, transposed: recipeT[n, t]
            st_sb = constp.tile([NN, T], BF16, tag="st")
            for tt in range(TT):
                stp = psums.tile([NN, P], BF16, tag="stp")
                nc.tensor.transpose(stp[:], s_red[:, tt, :], identb)
                nc.vector.tensor_copy(st_sb[:, ts(tt, P)], stp[:])
            rt_ps = psums.tile([NB, T], F32, tag="rtps")
            nc.tensor.matmul(rt_ps[:], recs[:], st_sb[:], start=True, stop=True)
            recipeT = constp.tile([NB, T], BF16, tag="recipeT")
            nc.vector.tensor_copy(recipeT[:], rt_ps[:])

            # RepR[(n,r), t] = SIGR * recipeT[n, t] replicated over r.
            # PSUM->SBUF copies split between Act and DVE.
            repr_sb = []
            for i in range(NRT):
                rp = psums.tile([P, T], F32, tag="rtps", name=f"rp{i}")
                nc.tensor.matmul(rp[:], blobb[:NB, BB_SEL + i * P:BB_SEL + (i + 1) * P],
                                 recipeT[:], start=True, stop=True)
                rr = constp.tile([P, T], BF16, tag=f"repr{i}", name=f"repr{i}")
                if i % 2 == 0:
                    nc.scalar.activation(rr[:], rp[:], AF.Copy)
                else:
                    nc.vector.tensor_copy(rr[:], rp[:])
                repr_sb.append(rr)
            filler("b", 8)

            # ---- WYT = YT * RepR;  rh = 32768*h via one accumulated matmul
            # (M folds the n-sum AND the r-replication: M[p',p] =
            #  32768*[p'%32 == p%32]) ----
            rh_ps = psums.tile([P, T], F32, tag="rhps")
            wyt = [mid.tile([P, T], BF16, tag=f"mid{i}", name=f"wyt{i}")
                   for i in range(NRT)]
            for i in range(NRT):
                nc.vector.tensor_mul(out=wyt[i][:], in0=yt_ps[i][:],
                                     in1=repr_sb[i][:])
                nc.tensor.matmul(rh_ps[:], qm, wyt[i][:],
                                 start=(i == 0), stop=(i == NRT - 1))
            filler("c", 12)

            # ---- CT = RepH * RepR (fp8): one Act copy of rh to bf16, then
            # all-SBUF bf16 muls on the DVE at 2x rate ----
            rh_sb = constp.tile([P, T], BF16, tag="rhsb")
            nc.scalar.activation(rh_sb[:], rh_ps[:], AF.Copy)
            ct8 = constp.tile([P, NRT, T], FP8, tag="ct8")
            with nc.allow_low_precision("ct is alpha-damped, fp8 is enough"):
                for i in range(NRT):
                    nc.vector.tensor_mul(out=ct8[:, i, :], in0=rh_sb[:],
                                         in1=repr_sb[i][:])
            # xf = 8192*(x + alpha*delta): psum copies split Act/DVE,
            # bf16 adds on DVE (2x rate), in place over xtb
            for half in range(2):
                dts = range(half * 4, half * 4 + 4)
                dl_ps = {dt: psum.tile([P, T], F32, tag="ps", name=f"dl{dt}")
                         for dt in dts}
                # j outer so the first delta matmuls only need ct8[0:2]
                for j in range(NRT // 2):
                    for dt in dts:
                        nc.tensor.matmul(dl_ps[dt][:],
                                         a2[:, 2 * j:2 * j + 2, ts(dt, P)],
                                         ct8[:, 2 * j:2 * j + 2, :],
                                         start=(j == 0), stop=(j == NRT // 2 - 1),
                                         perf_mode=DR)
                for dt in dts:
                    dl_sb = tmpp.tile([P, T], BF16, tag="dl", name=f"dl_sb{dt}")
                    if dt % 2 == 0:
                        nc.scalar.activation(dl_sb[:], dl_ps[dt][:], AF.Copy)
                    else:
                        nc.vector.tensor_copy(dl_sb[:], dl_ps[dt][:])
                    nc.vector.tensor_add(out=xtb[:, dt, :], in0=dl_sb[:],
                                         in1=xtb[:, dt, :])
            filler("e", 12)

            # ---- FFN up + exact gelu (descale 2^-13 folded into Act) ----
            g = constp.tile([P, FT, T], BF16, tag="g")
            for ftp in range(FT // 2):
                wu = stream.tile([P, 2, DC, P], BF16, tag="wu", name=f"wu{ftp}")
                nc.sync.dma_start(wu[:], wu_d[ftp])
                for j in range(2):
                    ft = 2 * ftp + j
                    u_ps = psum.tile([P, T], F32, tag="ps", name=f"u{ft}")
                    for dc in range(DC):
                        nc.tensor.matmul(u_ps[:], wu[:, j, dc, :], xtb[:, dc, :],
                                         start=(dc == 0), stop=(dc == DC - 1))
                    nc.scalar.activation(g[:, ft, :], u_ps[:], AF.Gelu,
                                         bias=bu[:, ft:ft + 1], scale=GS)

            # ---- FFN down + bias ----
            for dt in range(DT):
                o_ps = psum.tile([P, T], F32, tag="ps", name=f"o{dt}")
                for h in range(2):
                    wd = wdstream.tile([P, FT // 2, P], BF16, tag="wd",
                                       name=f"wd{dt}_{h}")
                    nc.scalar.dma_start(wd[:], wd_d[dt * 2 + h])
                    for fc in range(FT // 2):
                        fcg = h * (FT // 2) + fc
                        nc.tensor.matmul(o_ps[:], wd[:, fc, :], g[:, fcg, :],
                                         start=(fcg == 0), stop=(fcg == FT - 1))
                ot = otp.tile([P, T], F32, tag="ot", name=f"ot{dt}")
                nc.vector.tensor_scalar_add(ot[:], o_ps[:], bd[:, dt:dt + 1])
                nc.sync.dma_start(out_d[:, dt, :], ot[:])

    nc.finalize()
    return nc


def _get_nc():
    if _BUILT[0] is None:
        _BUILT[0] = _build_nc()
    return _BUILT[0]


def kernel(x, neuron_idx, neuron_weights, neuron_recipe, basis_A,
           w_up_w, w_up_b, w_down_w, w_down_b, alpha):
    import ml_dtypes
    nc = _get_nc()
    bf16 = ml_dtypes.bfloat16
    fp8 = ml_dtypes.float8_e4m3  # trn2 fp8e4: IEEE-style, max normal 240

    def to8(a):
        return np.clip(a, -F8MAX, F8MAX).astype(fp8)

    x = np.asarray(x, dtype=np.float32).reshape(NCORES * T, D)
    idxf = np.asarray(neuron_idx).astype(np.float32).reshape(NCORES * T, K)
    wgt = np.asarray(neuron_weights, dtype=np.float32).reshape(NCORES * T, K)
    rec = np.asarray(neuron_recipe, dtype=np.float32)
    bA = np.asarray(basis_A, dtype=np.float32)
    wu = np.asarray(w_up_w, dtype=np.float32)
    bu_in = np.asarray(w_up_b, dtype=np.float32)
    wd = np.asarray(w_down_w, dtype=np.float32)
    bd_in = np.asarray(w_down_b, dtype=np.float32)
    alpha_f = float(np.asarray(alpha, dtype=np.float32))

    # replicated operands, packed into the on-device layouts
    a1 = to8(np.ascontiguousarray(
        bA.transpose(1, 0, 2).reshape(D, NB * R)
        .reshape(DC, P, NB * R).transpose(1, 0, 2)) * SA)
    a2 = to8(np.ascontiguousarray(
        bA.transpose(0, 2, 1).reshape(NB * R, D)
        .reshape(NRT, P, D).transpose(1, 0, 2)) * (S2 * alpha_f))
    wu_p = np.ascontiguousarray(
        wu.reshape(DC, P, FT // 2, 2, P).transpose(2, 1, 3, 0, 4)
    ).astype(bf16)
    wd_p = np.ascontiguousarray(
        wd.reshape(2, FT // 2, P, DT, P).transpose(3, 0, 2, 1, 4)
        .reshape(DT * 2, P, FT // 2, P)).astype(bf16)

    blobf = np.zeros((P, BF_W), dtype=np.float32)
    blobf[:, BF_BU:BF_BU + FT] = bu_in.reshape(FT, P).T
    blobf[:, BF_BD:BF_BD + DT] = bd_in.reshape(DT, P).T

    blobb_base = np.zeros((P, BB_W), dtype=np.float32)
    blobb_base[:, BB_IOTA:BB_IOTA + NN * K] = np.repeat(
        np.arange(NN, dtype=np.float32), K)[None, :]
    blobb_base[:, BB_ID:BB_ID + P] = np.eye(P, dtype=np.float32)
    # SEL[n, i*128+m] = SIGR iff n == 4i + m//32
    for n in range(NB):
        i, nloc = divmod(n, NRT)
        blobb_base[n, BB_SEL + i * P + nloc * R: BB_SEL + i * P + (nloc + 1) * R] = SIGR
    blobb_base[:, BB_QM:BB_QM + P] = SIGM * (
        np.arange(P)[:, None] % R == np.arange(P)[None, :] % R)
    blobb_base[:NN, BB_REC:BB_REC + NB] = rec

    shared = {
        "blobf": blobf,
        "a1": a1, "a2": a2, "wu": wu_p, "wd": wd_p,
    }
    in_maps = []
    idxw = np.concatenate([idxf, wgt], axis=1)  # [N*T, 16]
    for c in range(NCORES):
        xc = x[c * T:(c + 1) * T]  # [T, D]
        xtc = np.ascontiguousarray(xc.T.reshape(DC, P, T).transpose(1, 0, 2))
        blobb = blobb_base.copy()
        blobb[:, BB_IDX:BB_IDX + 2 * K * TT] = (
            idxw[c * T:(c + 1) * T].reshape(TT, P, 2 * K).transpose(1, 0, 2)
            .reshape(P, 2 * K * TT))
        in_maps.append({"x8": to8(xtc * SX), "xtb": (xtc * XS).astype(bf16),
                        "blobb": blobb.astype(bf16), **shared})

    res = run_bass_kernel_spmd(nc, in_maps, core_ids=list(range(NCORES)))

    out = np.empty((NCORES * T, D), dtype=np.float32)
    for c in range(NCORES):
        ot = res.results[c]["outT"]  # [P, DT, T]
        out[c * T:(c + 1) * T] = ot.transpose(1, 0, 2).reshape(D, T).T
    return out.reshape(2, 2048, D)


# revision 16
# speedup vs baseline: 1.0690x; 1.0057x over previous
"""BasisResidualFFN Trainium2 kernel.

Math (per token t):
  recipe_soft = softmax(neuron_recipe, axis=-1)                 [64, 16]
  tr[t, :]    = sum_k w[t,k] * recipe_soft[idx[t,k], :]         [16]
  Y[t, (n,r)] = sum_d x[t,d] * basis_A[n,d,r]
  h[t, r]     = sum_n tr[t,n] * Y[t,(n,r)]
  delta[t, d] = sum_{n,r} basis_A[n,d,r] * tr[t,n] * h[t,r]
  out         = gelu((x + alpha*delta) @ w_up + b_up) @ w_down + b_down

Distribution: pure data parallel. B*S = 4096 tokens sharded 512/core
across 8 NeuronCores; all weights replicated. Everything on device is
computed feature-major (features on partitions, tokens on the free
axis, 512 tokens per matmul) so no on-device activation transposes are
needed anywhere in the FFN; x arrives pre-transposed from the host and
the output is un-transposed on the host.

Precision: the FFN runs bf16 (it dominates the error budget). The
basis/routing path runs fp8e4 with DoubleRow matmuls (2x PE throughput
when the HAM clock is at 8/8) -- its errors enter the output only
through alpha*delta with alpha ~ 0.1, so they are strongly damped
(measured 0.40% rel err end to end vs 0.345% for all-bf16).

fp8 scale chain (all powers of two, folded into host constants):
  x8 = 16*x, a1 = 256*A1  ->  YT_psum = 4096*Y
  SEL *= 2^-12            ->  repr = tr/4096, wyt = Y*tr (bf16)
  M = 32768*[p'%32==p%32] ->  rh_psum = sum_i M^T wyt_i = 32768*h
                              (ht and its copy are folded into one
                               accumulated matmul)
  ct = rh (.) repr = 8*tr*h  (fp8, |ct| < 26 << 240 = trn fp8e4 max)
  a2 = 1024*alpha*A2      ->  dl_psum = 8192*alpha*delta
  xtb = 8192*x (bf16)     ->  xf = Copy(dl) + xtb = 8192*(x+a*delta)
  gelu(u) evaluated as Act(scale=2^-13) on u_psum = 8192*u.

Scheduling: PE is warmed with matmuls on a memset tile (no DMA
dependency) from ~8us, and small filler matmuls keep the HAM clock
gate at 8/8 through the DVE-bound stretches of the basis phase (an
idle PE is re-throttled to half clock, which would double the cost of
everything after). PSUM->SBUF copies are split between the Act engine
and the DVE so neither serializes the chain. The two HWDGE DMA rings
are loaded in strict need-order (each dma_start costs ~2us fixed +
bytes/rate, FIFO per ring), with all small constants merged into one
per-core blob.
"""

import numpy as np

import concourse.bass as bass
import concourse.mybir as mybir
import concourse.tile as tile
from concourse import bacc
from concourse.bass import ts
from concourse.bass_utils import run_bass_kernel_spmd

P = 128
NCORES = 8
T = 512            # tokens per core
D = 1024
DFF = 4096
NB = 16            # n_basis
R = 32             # rank
NN = 64            # n_neurons
K = 8              # top-k
DC = D // P        # 8 contraction chunks over d
FT = DFF // P      # 32 ff tiles
DT = D // P        # 8 output d tiles
NRT = (NB * R) // P  # 4 (n,r) tiles
TT = T // P        # 4 token tiles per core

# fp8 scale chain (powers of two)
SX = 16.0          # x fp8 scale
SA = 256.0         # a1 fp8 scale
S2 = 1024.0        # alpha*a2 fp8 scale
SIGR = 2.0 ** -12  # SEL scale  (= 1/(SX*SA))
SIGM = 32768.0     # M scale    (SIGM*SIGR = 8 = ct scale)
XS = 8192.0        # xtb prescale (= ct_scale * S2)
GS = 2.0 ** -13    # gelu input scale (= 1/XS)
F8MAX = 240.0      # trn2 fp8e4 max normal

# merged bf16 blob column layout (per-core: contains idx/weights)
BB_IDX = 0
BB_IOTA = BB_IDX + 2 * K * TT      # 64
BB_ID = BB_IOTA + NN * K           # 576
BB_SEL = BB_ID + P                 # 704
BB_QM = BB_SEL + NRT * P           # 1216
BB_REC = BB_QM + P                 # 1344
BB_W = BB_REC + NB                 # 1360
BF_BU, BF_BD, BF_W = 0, 32, 40     # f32 biases

F32 = mybir.dt.float32
BF16 = mybir.dt.bfloat16
FP8 = mybir.dt.float8e4

NWARM = 13

_BUILT = [None]


def _build_nc():
    nc = bacc.Bacc(None, target_bir_lowering=False)

    x8_d = nc.dram_tensor("x8", [P, DC, T], FP8, kind="ExternalInput")
    xtb_d = nc.dram_tensor("xtb", [P, DC, T], BF16, kind="ExternalInput")
    blobb_d = nc.dram_tensor("blobb", [P, BB_W], BF16, kind="ExternalInput")
    blobf_d = nc.dram_tensor("blobf", [P, BF_W], F32, kind="ExternalInput")
    a1_d = nc.dram_tensor("a1", [P, DC, NB * R], FP8, kind="ExternalInput")
    a2_d = nc.dram_tensor("a2", [P, NRT, D], FP8, kind="ExternalInput")
    wu_d = nc.dram_tensor("wu", [FT // 2, P, 2, DC, P], BF16, kind="ExternalInput")
    wd_d = nc.dram_tensor("wd", [DT * 2, P, FT // 2, P], BF16, kind="ExternalInput")
    out_d = nc.dram_tensor("outT", [P, DT, T], F32, kind="ExternalOutput")

    AX = mybir.AxisListType.X
    AF = mybir.ActivationFunctionType
    ALU = mybir.AluOpType
    DR = mybir.MatmulPerfMode.DoubleRow

    with tile.TileContext(nc) as tc:
        with (
            tc.tile_pool(name="const", bufs=1) as constp,
            tc.tile_pool(name="stream", bufs=4) as stream,
            tc.tile_pool(name="otp", bufs=3) as otp,
            tc.tile_pool(name="wdstream", bufs=4) as wdstream,
            tc.tile_pool(name="mid", bufs=1) as mid,
            tc.tile_pool(name="small", bufs=2) as small,
            tc.tile_pool(name="tmpp", bufs=3) as tmpp,
            tc.tile_pool(name="psum", bufs=4, space="PSUM") as psum,
            tc.tile_pool(name="psums", bufs=1, space="PSUM") as psums,
        ):
            # ---- PE warm-up on a memset tile: no DMA dependency, so the
            # HAM clock gate ramps from ~8us while the input DMAs land ----
            wz = constp.tile([P, T + P], BF16, tag="wz")
            nc.gpsimd.memset(wz[:], 0.0)
            warm_ps = psums.tile([P, T], F32, tag="htps", name="warm")
            for w in range(NWARM):
                nc.tensor.matmul(warm_ps[:], wz[:, :P], wz[:, :T],
                                 start=(w == 0), stop=(w == NWARM - 1))

            def filler(name, n):
                # small matmuls that keep the HAM clock gate at 8/8 while
                # the PE waits on DVE stages; drained in ~110ns each once
                # real work unblocks.  Anchored via the idle Act engine.
                f_ps = psums.tile([P, T], F32, tag="htps", name=f"fill_{name}")
                for i in range(n):
                    nc.tensor.matmul(f_ps[:, :P], wz[:, :P], wz[:, T:T + P],
                                     start=(i == 0), stop=(i == n - 1))
                fa = tmpp.tile([P, 1], F32, tag="fanchor", name=f"fa_{name}")
                nc.scalar.activation(fa[:], f_ps[:, 0:1], AF.Copy)

            # ---- resident loads: strict need-order FIFO on the two HWDGE
            # rings so critical-path tensors never starve behind bulk ----
            # sync ring: routing blob first (gates the DVE scatter chain),
            # then x8 (YT), a2, biases, then the wu stream
            blobb = constp.tile([P, BB_W], BF16, tag="blobb")
            nc.sync.dma_start(blobb[:], blobb_d[:])
            x8 = constp.tile([P, DC, T], FP8, tag="x8")
            nc.sync.dma_start(x8[:], x8_d[:])
            a2 = constp.tile([P, NRT, D], FP8, tag="a2")
            nc.sync.dma_start(a2[:], a2_d[:])
            blobf = constp.tile([P, BF_W], F32, tag="blobf")
            nc.sync.dma_start(blobf[:], blobf_d[:])
            # scalar ring: a1 (YT stationary), then xtb, later the wd stream
            a1 = constp.tile([P, DC, NB * R], FP8, tag="a1")
            nc.scalar.dma_start(a1[:], a1_d[:])
            xtb = constp.tile([P, DC, T], BF16, tag="xtb")
            nc.scalar.dma_start(xtb[:], xtb_d[:])

            bu = blobf[:, BF_BU:BF_BU + FT]
            bd = blobf[:, BF_BD:BF_BD + DT]
            identb = blobb[:, BB_ID:BB_ID + P]
            qm = blobb[:, BB_QM:BB_QM + P]
            rec = blobb[:NN, BB_REC:BB_REC + NB]

            # ---- routing: weighted one-hot scatter S[t, neuron], batched
            # over all 4 token tiles (all-bf16 for 2x DVE); the K-reduction
            # is split DVE/GpSimd ----
            iota_b = blobb[:, BB_IOTA:BB_IOTA + NN * K].rearrange(
                "p (o n k) -> p o n k", o=1, k=K).to_broadcast((P, TT, NN, K))
            idxw = blobb[:, BB_IDX:BB_IDX + 2 * K * TT].rearrange(
                "p (t k) -> p t k", t=TT)
            idx_b = idxw[:, :, 0:K].rearrange(
                "p t (o k) -> p t o k", o=1).to_broadcast((P, TT, NN, K))
            w_b = idxw[:, :, K:2 * K].rearrange(
                "p t (o k) -> p t o k", o=1).to_broadcast((P, TT, NN, K))
            sk = small.tile([P, TT, NN, K], BF16, tag="sk")
            nc.vector.tensor_tensor(sk[:], iota_b, idx_b, ALU.is_equal)
            nc.vector.tensor_tensor(sk[:], sk[:], w_b, ALU.mult)
            s_red = small.tile([P, TT, NN], BF16, tag="sred")
            with nc.allow_low_precision("s values are sums of <=8 weights"):
                nc.vector.reduce_sum(s_red[:, 0:2, :], sk[:, 0:2, :, :], axis=AX)
                # GpSimd takes the other half via pairwise adds (it has no
                # free-axis reduce)
                nc.gpsimd.tensor_tensor(sk[:, 2:4, :, 0:4], sk[:, 2:4, :, 0:4],
                                        sk[:, 2:4, :, 4:8], ALU.add)
                nc.gpsimd.tensor_tensor(sk[:, 2:4, :, 0:2], sk[:, 2:4, :, 0:2],
                                        sk[:, 2:4, :, 2:4], ALU.add)
                nc.gpsimd.tensor_tensor(s_red[:, 2:4, :], sk[:, 2:4, :, 0],
                                        sk[:, 2:4, :, 1], ALU.add)

            # ---- softmax over the 16-basis axis of the recipe table ----
            esb = small.tile([NN, NB], F32, tag="esb")
            ssum = small.tile([NN, 1], F32, tag="ssum")
            nc.scalar.activation(esb[:], rec, AF.Exp, accum_out=ssum[:])
            rsum = small.tile([NN, 1], F32, tag="rsum")
            nc.vector.reciprocal(rsum[:], ssum[:])
            recs = constp.tile([NN, NB], BF16, tag="recs")
            nc.scalar.activation(recs[:], esb[:], AF.Copy, scale=rsum[:, 0:1])

            # anchor read keeps the warm-up matmuls from being dead-code
            # eliminated (Act engine; DVE is the busy one here)
            warm_anchor = tmpp.tile([P, 1], F32, tag="fanchor", name="wanchor")
            nc.scalar.activation(warm_anchor[:], warm_ps[:, 0:1], AF.Copy)

            # ---- YT = A1^T @ xT, fp8 DoubleRow (2 d-chunks per matmul),
            # with the small routing matmuls interleaved between YT tiles so
            # neither chain waits for the other on the in-order PE queue ----
            def yt_tile(i):
                for cp in range(DC // 2):
                    nc.tensor.matmul(yt_ps[i][:],
                                     a1[:, 2 * cp:2 * cp + 2, ts(i, P)],
                                     x8[:, 2 * cp:2 * cp + 2, :],
                                     start=(cp == 0), stop=(cp == DC // 2 - 1),
                                     perf_mode=DR)

            yt_ps = [psum.tile([P, T], F32, tag="ps", name=f"yt{i}")
                     for i in range(NRT)]
            yt_tile(0)

            # scatter transposes + token recipes, transposed: recipeT[n, t]
            st_sb = constp.tile([NN, T], BF16, tag="st")
            for tt in range(TT):
                # alternate between the two serial psum slots for ping-pong
                stp = psums.tile([NN, P], BF16, tag="rhps" if tt % 2 else "rtps",
                                 name=f"stp{tt}")
                nc.tensor.transpose(stp[:], s_red[:, tt, :], identb)
                nc.vector.tensor_copy(st_sb[:, ts(tt, P)], stp[:])
            rt_ps = psums.tile([NB, T], F32, tag="rtps")
            nc.tensor.matmul(rt_ps[:], recs[:], st_sb[:], start=True, stop=True)
            recipeT = constp.tile([NB, T], BF16, tag="recipeT")
            nc.vector.tensor_copy(recipeT[:], rt_ps[:])

            yt_tile(1)

            # RepR[(n,r), t] = SIGR * recipeT[n, t] replicated over r.
            # PSUM->SBUF copies split between Act and DVE.
            repr_sb = []
            for i in range(NRT):
                rp = psums.tile([P, T], F32, tag="rtps", name=f"rp{i}")
                nc.tensor.matmul(rp[:], blobb[:NB, BB_SEL + i * P:BB_SEL + (i + 1) * P],
                                 recipeT[:], start=True, stop=True)
                rr = constp.tile([P, T], BF16, tag=f"repr{i}", name=f"repr{i}")
                if i % 2 == 0:
                    nc.scalar.activation(rr[:], rp[:], AF.Copy)
                else:
                    nc.vector.tensor_copy(rr[:], rp[:])
                repr_sb.append(rr)

            yt_tile(2)
            yt_tile(3)
            filler("a", 8)

            # ---- WYT = YT * RepR;  rh = 32768*h via one accumulated matmul
            # (M folds the n-sum AND the r-replication: M[p',p] =
            #  32768*[p'%32 == p%32]) ----
            rh_ps = psums.tile([P, T], F32, tag="rhps")
            wyt = [mid.tile([P, T], BF16, tag=f"mid{i}", name=f"wyt{i}")
                   for i in range(NRT)]
            for i in range(NRT):
                nc.vector.tensor_mul(out=wyt[i][:], in0=yt_ps[i][:],
                                     in1=repr_sb[i][:])
                nc.tensor.matmul(rh_ps[:], qm, wyt[i][:],
                                 start=(i == 0), stop=(i == NRT - 1))
            filler("c", 12)

            # ---- CT = RepH * RepR (fp8): one Act copy of rh to bf16, then
            # all-SBUF bf16 muls on the DVE at 2x rate ----
            rh_sb = constp.tile([P, T], BF16, tag="rhsb")
            nc.scalar.activation(rh_sb[:], rh_ps[:], AF.Copy)
            ct8 = constp.tile([P, NRT, T], FP8, tag="ct8")
            with nc.allow_low_precision("ct is alpha-damped, fp8 is enough"):
                for i in range(NRT):
                    nc.vector.tensor_mul(out=ct8[:, i, :], in0=rh_sb[:],
                                         in1=repr_sb[i][:])
            # xf = 8192*(x + alpha*delta): psum copies split Act/DVE,
            # bf16 adds on DVE (2x rate), in place over xtb
            for half in range(2):
                dts = range(half * 4, half * 4 + 4)
                dl_ps = {dt: psum.tile([P, T], F32, tag="ps", name=f"dl{dt}")
                         for dt in dts}
                # j outer so the first delta matmuls only need ct8[0:2]
                for j in range(NRT // 2):
                    for dt in dts:
                        nc.tensor.matmul(dl_ps[dt][:],
                                         a2[:, 2 * j:2 * j + 2, ts(dt, P)],
                                         ct8[:, 2 * j:2 * j + 2, :],
                                         start=(j == 0), stop=(j == NRT // 2 - 1),
                                         perf_mode=DR)
                for dt in dts:
                    dl_sb = tmpp.tile([P, T], BF16, tag="dl", name=f"dl_sb{dt}")
                    if dt % 2 == 0:
                        nc.scalar.activation(dl_sb[:], dl_ps[dt][:], AF.Copy)
                        nc.vector.tensor_add(out=xtb[:, dt, :], in0=dl_sb[:],
                                             in1=xtb[:, dt, :])
                    else:
                        nc.vector.tensor_copy(dl_sb[:], dl_ps[dt][:])
                        nc.gpsimd.tensor_tensor(xtb[:, dt, :], dl_sb[:],
                                                xtb[:, dt, :], ALU.add)
            filler("e", 16)

            # ---- FFN up + exact gelu (descale 2^-13 folded into Act) ----
            g = constp.tile([P, FT, T], BF16, tag="g")
            for ftp in range(FT // 2):
                wu = stream.tile([P, 2, DC, P], BF16, tag="wu", name=f"wu{ftp}")
                nc.sync.dma_start(wu[:], wu_d[ftp])
                for j in range(2):
                    ft = 2 * ftp + j
                    u_ps = psum.tile([P, T], F32, tag="ps", name=f"u{ft}")
                    for dc in range(DC):
                        nc.tensor.matmul(u_ps[:], wu[:, j, dc, :], xtb[:, dc, :],
                                         start=(dc == 0), stop=(dc == DC - 1))
                    nc.scalar.activation(g[:, ft, :], u_ps[:], AF.Gelu,
                                         bias=bu[:, ft:ft + 1], scale=GS)

            # ---- FFN down + bias ----
            for dt in range(DT):
                o_ps = psum.tile([P, T], F32, tag="ps", name=f"o{dt}")
                for h in range(2):
                    wd = wdstream.tile([P, FT // 2, P], BF16, tag="wd",
                                       name=f"wd{dt}_{h}")
                    nc.scalar.dma_start(wd[:], wd_d[dt * 2 + h])
                    for fc in range(FT // 2):
                        fcg = h * (FT // 2) + fc
                        nc.tensor.matmul(o_ps[:], wd[:, fc, :], g[:, fcg, :],
                                         start=(fcg == 0), stop=(fcg == FT - 1))
                ot = otp.tile([P, T], F32, tag="ot", name=f"ot{dt}")
                nc.vector.tensor_scalar_add(ot[:], o_ps[:], bd[:, dt:dt + 1])
                nc.sync.dma_start(out_d[:, dt, :], ot[:])

    nc.finalize()
    return nc


def _get_nc():
    if _BUILT[0] is None:
        _BUILT[0] = _build_nc()
    return _BUILT[0]


def kernel(x, neuron_idx, neuron_weights, neuron_recipe, basis_A,
           w_up_w, w_up_b, w_down_w, w_down_b, alpha):
    import ml_dtypes
    nc = _get_nc()
    bf16 = ml_dtypes.bfloat16
    fp8 = ml_dtypes.float8_e4m3  # trn2 fp8e4: IEEE-style, max normal 240

    def to8(a):
        return np.clip(a, -F8MAX, F8MAX).astype(fp8)

    x = np.asarray(x, dtype=np.float32).reshape(NCORES * T, D)
    idxf = np.asarray(neuron_idx).astype(np.float32).reshape(NCORES * T, K)
    wgt = np.asarray(neuron_weights, dtype=np.float32).reshape(NCORES * T, K)
    rec = np.asarray(neuron_recipe, dtype=np.float32)
    bA = np.asarray(basis_A, dtype=np.float32)
    wu = np.asarray(w_up_w, dtype=np.float32)
    bu_in = np.asarray(w_up_b, dtype=np.float32)
    wd = np.asarray(w_down_w, dtype=np.float32)
    bd_in = np.asarray(w_down_b, dtype=np.float32)
    alpha_f = float(np.asarray(alpha, dtype=np.float32))

    # replicated operands, packed into the on-device layouts
    a1 = to8(np.ascontiguousarray(
        bA.transpose(1, 0, 2).reshape(D, NB * R)
        .reshape(DC, P, NB * R).transpose(1, 0, 2)) * SA)
    a2 = to8(np.ascontiguousarray(
        bA.transpose(0, 2, 1).reshape(NB * R, D)
        .reshape(NRT, P, D).transpose(1, 0, 2)) * (S2 * alpha_f))
    wu_p = np.ascontiguousarray(
        wu.reshape(DC, P, FT // 2, 2, P).transpose(2, 1, 3, 0, 4)
    ).astype(bf16)
    wd_p = np.ascontiguousarray(
        wd.reshape(2, FT // 2, P, DT, P).transpose(3, 0, 2, 1, 4)
        .reshape(DT * 2, P, FT // 2, P)).astype(bf16)

    blobf = np.zeros((P, BF_W), dtype=np.float32)
    blobf[:, BF_BU:BF_BU + FT] = bu_in.reshape(FT, P).T
    blobf[:, BF_BD:BF_BD + DT] = bd_in.reshape(DT, P).T

    blobb_base = np.zeros((P, BB_W), dtype=np.float32)
    blobb_base[:, BB_IOTA:BB_IOTA + NN * K] = np.repeat(
        np.arange(NN, dtype=np.float32), K)[None, :]
    blobb_base[:, BB_ID:BB_ID + P] = np.eye(P, dtype=np.float32)
    # SEL[n, i*128+m] = SIGR iff n == 4i + m//32
    for n in range(NB):
        i, nloc = divmod(n, NRT)
        blobb_base[n, BB_SEL + i * P + nloc * R: BB_SEL + i * P + (nloc + 1) * R] = SIGR
    blobb_base[:, BB_QM:BB_QM + P] = SIGM * (
        np.arange(P)[:, None] % R == np.arange(P)[None, :] % R)
    blobb_base[:NN, BB_REC:BB_REC + NB] = rec

    shared = {
        "blobf": blobf,
        "a1": a1, "a2": a2, "wu": wu_p, "wd": wd_p,
    }
    in_maps = []
    idxw = np.concatenate([idxf, wgt], axis=1)  # [N*T, 16]
    for c in range(NCORES):
        xc = x[c * T:(c + 1) * T]  # [T, D]
        xtc = np.ascontiguousarray(xc.T.reshape(DC, P, T).transpose(1, 0, 2))
        blobb = blobb_base.copy()
        blobb[:, BB_IDX:BB_IDX + 2 * K * TT] = (
            idxw[c * T:(c + 1) * T].reshape(TT, P, 2 * K).transpose(1, 0, 2)
            .reshape(P, 2 * K * TT))
        in_maps.append({"x8": to8(xtc * SX), "xtb": (xtc * XS).astype(bf16),
                        "blobb": blobb.astype(bf16), **shared})

    res = run_bass_kernel_spmd(nc, in_maps, core_ids=list(range(NCORES)))

    out = np.empty((NCORES * T, D), dtype=np.float32)
    for c in range(NCORES):
        ot = res.results[c]["outT"]  # [P, DT, T]
        out[c * T:(c + 1) * T] = ot.transpose(1, 0, 2).reshape(D, T).T
    return out.reshape(2, 2048, D)
